# revision 9
# baseline (speedup 1.0000x reference)
"""Trainium2 Bass kernel for nn_Attention_35021163332119.

Full multi-head attention: qkv = x @ w_qkv; RoPE(q, k); softmax(q k^T / sqrt(dh)) v;
out = heads @ w_out + b_out.  B=2, N=2048, DIM=1024, H=16, DH=64.

Sharding: 8 cores = (batch b in {0,1}) x (head-group g in {0..3} of 4 heads).
Each core computes its 4 heads end-to-end plus the partial output projection
for its head-group's rows of w_out; the host sums the 4 partials per batch
(in fp32, from fp16 device partials) and adds b_out.

On-core layout: x is host-transposed to xT [DIM, N] so the contraction dim
sits on SBUF partitions.  q,k are produced transposed ([dh, n], head pairs
stacked on 128 partitions) straight out of the QKV matmul; v is produced in
natural [n, dh] layout with an extra ones column, so the PV matmul (M=65)
also accumulates the softmax denominator in row 64.  RoPE's interleaved
pair-rotation is a 128x128 +/-1 permutation matmul on the PE plus DVE
multiplies against cos/sin tables.

Everything on the probability/QK side runs in bf16 (validated 7e-3 rel err
vs the 2e-2 gate): bf16 weights enable fast weight load on the PE and 2x/4x
DVE modes for the RoPE elementwise work.  Input DMA is stripe-major (weights
first, then xT in 512-column stripes across all k-tiles) so the first QKV
matmul starts as soon as ~1.5 MB has landed.  The second head-pair's QKV
matmuls and the output projection are interleaved into the attention phase
to fill PE slack under the scalar-engine exp stream.  Partial outputs DMA
out as fp16.
"""

import numpy as np

B, N, DIM, H, DH = 2, 2048, 1024, 16, 64
ROPE_BASE = 10000.0
SCALE = DH ** -0.5
N_CORES = 8
G = 4                 # heads per core
KT = DIM // 128       # contraction tiles
NT = N // 128         # sequence tiles

_cache = {}


def _rope_tables():
    inv_freq = (1.0 / (ROPE_BASE ** (np.arange(0, DH, 2, dtype=np.float32) / DH)))
    t = np.arange(N, dtype=np.float32)
    freqs = t[:, None] * inv_freq[None, :]          # [N, DH/2]
    freqs = np.repeat(freqs, 2, axis=-1)            # [N, DH] interleaved
    cosT = np.cos(freqs).T.astype(np.float32)       # [DH, N]
    sinT = np.sin(freqs).T.astype(np.float32)
    cos2 = np.concatenate([cosT, cosT], axis=0)     # [128, N] two heads stacked
    sin2 = np.concatenate([sinT, sinT], axis=0)
    return np.ascontiguousarray(cos2), np.ascontiguousarray(sin2)


def _p2t():
    # rot = P2 @ qT with P2 = blockdiag(P, P), P[2t, 2t+1] = -1, P[2t+1, 2t] = 1
    # matmul computes lhsT.T @ rhs, so pass P2.T
    p = np.zeros((DH, DH), dtype=np.float32)
    for t in range(DH // 2):
        p[2 * t, 2 * t + 1] = -1.0
        p[2 * t + 1, 2 * t] = 1.0
    p2 = np.zeros((128, 128), dtype=np.float32)
    p2[:DH, :DH] = p
    p2[DH:, DH:] = p
    return np.ascontiguousarray(p2.T)


def _build():
    if "nc" in _cache:
        return _cache["nc"]

    import concourse.mybir as mybir
    import concourse.tile as tile
    from concourse import bacc

    F32 = mybir.dt.float32
    F16 = mybir.dt.float16
    BF16 = mybir.dt.bfloat16
    EXP = mybir.ActivationFunctionType.Exp

    nc = bacc.Bacc("TRN2", target_bir_lowering=False, debug=False)
    xT_d = nc.dram_tensor("xT", [DIM, N], BF16, kind="ExternalInput")
    wqk_d = nc.dram_tensor("wqk", [DIM, 4 * 128], BF16, kind="ExternalInput")
    wv_d = nc.dram_tensor("wv", [DIM, G * DH], BF16, kind="ExternalInput")
    wout_d = nc.dram_tensor("wout", [G * DH, DIM], BF16, kind="ExternalInput")
    cos_d = nc.dram_tensor("cos2", [128, N], BF16, kind="ExternalInput")
    sin_d = nc.dram_tensor("sin2", [128, N], BF16, kind="ExternalInput")
    p2t_d = nc.dram_tensor("p2t", [128, 128], BF16, kind="ExternalInput")
    part_d = nc.dram_tensor("part", [N, DIM], F16, kind="ExternalOutput")

    with tile.TileContext(nc) as tc:
        with tc.tile_pool(name="persist", bufs=1) as persist, \
             tc.tile_pool(name="att", bufs=8) as att, \
             tc.tile_pool(name="norm_w", bufs=2) as norm_w, \
             tc.tile_pool(name="outp", bufs=3) as outp, \
             tc.tile_pool(name="xph", bufs=1) as xph, \
             tc.tile_pool(name="rope_w", bufs=2) as rope_w, \
             tc.tile_pool(name="ps", bufs=3, space="PSUM") as ps, \
             tc.tile_pool(name="pso", bufs=2, space="PSUM") as pso:

            # ---- persistent tiles ----
            qk_sb = [persist.tile([128, N], BF16, tag=f"qk{m}", name=f"qk{m}")
                     for m in range(4)]          # q01T, q23T, k01T, k23T
            v_aug = persist.tile([128, NT, G, DH + 1], BF16, tag="vaug")
            wout_sb = [persist.tile([128, DIM], BF16, tag=f"wo{kk}", name=f"wo{kk}")
                       for kk in range(2)]
            outT = [persist.tile([128, N], BF16, tag=f"outT{p}", name=f"outT{p}")
                    for p in range(2)]

            # ---- phase-1 tiles ----
            xT = [xph.tile([128, N], BF16, tag=f"xT{k}", name=f"xT{k}")
                  for k in range(KT)]
            wqk = [xph.tile([128, 4 * 128], BF16, tag=f"wqk{k}", name=f"wqk{k}")
                   for k in range(KT)]
            wv = [xph.tile([128, G * DH], BF16, tag=f"wv{k}", name=f"wv{k}")
                  for k in range(KT)]
            cos2 = xph.tile([128, N], BF16, tag="cos2")
            sin2 = xph.tile([128, N], BF16, tag="sin2")
            p2t = xph.tile([128, 128], BF16, tag="p2t")
            ones_col = xph.tile([128, NT, G, 1], F32, tag="ones")

            # qk weights first, then xT stripe-major; tables and the
            # projection weights slot in behind the first stripe (they are
            # first needed at rope / projection time)
            for k in range(KT):
                nc.sync.dma_start(
                    out=wqk[k],
                    in_=wqk_d.ap().rearrange("(t p) m -> t p m", p=128)[k])
            nc.vector.memset(ones_col, 1.0)
            for c in range(4):
                csl = slice(c * 512, (c + 1) * 512)
                for k in range(KT):
                    nc.sync.dma_start(
                        out=xT[k][:, csl],
                        in_=xT_d.ap().rearrange(
                            "(t p) n -> t p n", p=128)[k][:, csl])
                if c == 0:
                    nc.sync.dma_start(out=p2t, in_=p2t_d.ap())
                    nc.sync.dma_start(out=sin2, in_=sin_d.ap())
                    nc.sync.dma_start(out=cos2, in_=cos_d.ap())
                elif c == 1:
                    for k in range(KT):
                        nc.sync.dma_start(
                            out=wv[k],
                            in_=wv_d.ap().rearrange(
                                "(t p) m -> t p m", p=128)[k])
                elif c == 2:
                    for kk in range(2):
                        nc.sync.dma_start(
                            out=wout_sb[kk],
                            in_=wout_d.ap().rearrange(
                                "(t p) m -> t p m", p=128)[kk])

            def qk_chunks(m, crange):
                # m: 0,1 = q01T,q23T; 2,3 = k01T,k23T.  crange indexes 1024-
                # wide chunks (two 512 halves per PSUM tile).  pair-0 evacs on
                # the scalar engine (idle during lead-in); pair-1 evacs on DVE
                # so the scalar engine stays pure-exp during attention.
                for c2 in crange:
                    mm = ps.tile([128, 1024], F32, tag="s", name="mm_qk")
                    for half in range(2):
                        hsl = slice(half * 512, (half + 1) * 512)
                        csl = slice(c2 * 1024 + half * 512,
                                    c2 * 1024 + (half + 1) * 512)
                        for k in range(KT):
                            nc.tensor.matmul(
                                mm[:, hsl],
                                wqk[k][:, m * 128:(m + 1) * 128],
                                xT[k][:, csl],
                                start=(k == 0), stop=(k == KT - 1))
                    osl = slice(c2 * 1024, (c2 + 1) * 1024)
                    if m in (0, 2):
                        nc.scalar.copy(qk_sb[m][:, osl], mm)
                    else:
                        nc.vector.tensor_copy(qk_sb[m][:, osl], mm)

            def rope_m(m):
                tmp = rope_w.tile([128, N], BF16, tag="ropetmp")
                for c2 in range(2):
                    rot = ps.tile([128, 1024], F32, tag="s", name="mm_rot")
                    for half in range(2):
                        csl = slice(c2 * 1024 + half * 512,
                                    c2 * 1024 + (half + 1) * 512)
                        nc.tensor.matmul(
                            rot[:, half * 512:(half + 1) * 512],
                            p2t, qk_sb[m][:, csl],
                            start=True, stop=True)
                    osl = slice(c2 * 1024, (c2 + 1) * 1024)
                    nc.vector.tensor_mul(tmp[:, osl], rot, sin2[:, osl])
                nc.vector.tensor_mul(qk_sb[m], qk_sb[m], cos2)
                nc.vector.tensor_add(qk_sb[m], qk_sb[m], tmp)

            def v_tiles(trange):
                for th in trange:           # one psum tile covers 4 n-tiles
                    mm = ps.tile([128, 1024], F32, tag="s", name="mm_v")
                    for quad in range(4):
                        tn = 4 * th + quad
                        for k in range(KT):
                            nc.tensor.matmul(
                                mm[:, quad * 256:(quad + 1) * 256],
                                xT[k][:, tn * 128:(tn + 1) * 128],
                                wv[k],
                                start=(k == 0), stop=(k == KT - 1))
                    nc.vector.tensor_copy(
                        v_aug[:, 4 * th:4 * th + 4, :, 0:DH],
                        mm.rearrange("p (t h d) -> p t h d", t=4, h=G))
                if trange and trange[-1] == NT // 4 - 1:
                    nc.vector.tensor_copy(v_aug[:, :, :, DH:DH + 1], ones_col)

            def attention(p, iq):
                """One (head-pair, i-quarter of 512) block.  exp of jj runs on
                the scalar engine while the PE does PV of jj-1."""
                qT = qk_sb[p]
                kTt = qk_sb[2 + p]
                i0 = iq * 512
                isl = slice(i0, i0 + 512)
                o_ps = [pso.tile([DH + 1, 512], F32, tag="o", name=f"o{hh}")
                        for hh in range(2)]

                def emit_pv(jj, exps):
                    for hh in range(2):
                        for half in range(2):
                            j = 2 * jj + half
                            nc.tensor.matmul(
                                o_ps[hh],
                                v_aug[:, j, 2 * p + hh, :],
                                exps[hh][:, half * 512:(half + 1) * 512],
                                start=(j == 0), stop=(j == NT - 1))

                pend = None
                for jj in range(NT // 2):
                    s_ps = [ps.tile([128, 1024], F32, tag="s", name=f"s{hh}")
                            for hh in range(2)]
                    for half in range(2):
                        j = 2 * jj + half
                        jsl = slice(j * 128, (j + 1) * 128)
                        for hh in range(2):
                            hsl = slice(hh * DH, (hh + 1) * DH)
                            nc.tensor.matmul(
                                s_ps[hh][:, half * 512:(half + 1) * 512],
                                kTt[hsl, jsl], qT[hsl, isl],
                                start=True, stop=True)
                    exps = []
                    for hh in range(2):
                        expT = att.tile([128, 1024], BF16, tag="exp")
                        nc.scalar.activation(expT, s_ps[hh], EXP, scale=SCALE)
                        exps.append(expT)
                    if pend is not None:
                        emit_pv(jj - 1, pend)
                    pend = exps
                emit_pv(NT // 2 - 1, pend)
                # evacuate PV accumulators so PSUM frees fast, then normalize
                # off the critical path
                for hh in range(2):
                    o_sb = norm_w.tile([DH + 1, 512], F32, tag=f"osb{hh}",
                                       name=f"osb{hh}")
                    nc.vector.tensor_copy(o_sb, o_ps[hh])
                    recip0 = norm_w.tile([1, 512], F32, tag=f"r0{hh}",
                                         name=f"r0{hh}")
                    nc.sync.dma_start(out=recip0, in_=o_sb[DH:DH + 1, :])
                    nc.vector.reciprocal_approx_fast(recip0, recip0)
                    bc = norm_w.tile([DH, 512], F32, tag=f"bc{hh}",
                                     name=f"bc{hh}")
                    nc.gpsimd.partition_broadcast(bc, recip0)
                    if hh == 0:
                        nc.vector.tensor_mul(outT[p][0:DH, isl],
                                             o_sb[0:DH, :], bc)
                    else:
                        tmpb = norm_w.tile([DH, 512], BF16, tag="tmpb")
                        nc.vector.tensor_mul(tmpb, o_sb[0:DH, :], bc)
                        nc.sync.dma_start(out=outT[p][DH:2 * DH, isl],
                                          in_=tmpb)

            def proj_tile(tn):
                nsl = slice(tn * 128, (tn + 1) * 128)
                f_ps = ps.tile([128, 1024], F32, tag="s", name="f_ps")
                for c2 in range(2):
                    c2sl = slice(c2 * 512, (c2 + 1) * 512)
                    for kk in range(2):
                        nc.tensor.matmul(
                            f_ps[:, c2sl],
                            outT[kk][:, nsl], wout_sb[kk][:, c2sl],
                            start=(kk == 0), stop=(kk == 1))
                out_sb = outp.tile([128, DIM], F16, tag="osb")
                nc.vector.tensor_copy(out_sb, f_ps)
                nc.sync.dma_start(
                    out=part_d.ap().rearrange("(t p) m -> t p m", p=128)[tn],
                    in_=out_sb)

            # ---- emission order ----
            # lead-in: k01 then q01 (pair 0) + v; pair 1 QKV and the output
            # projection interleave into the attention phase.
            qk_chunks(2, range(2))
            rope_m(2)
            qk_chunks(0, range(2))
            rope_m(0)
            v_tiles(range(NT // 4))
            attention(0, 0)
            qk_chunks(3, range(1))
            attention(0, 1)
            qk_chunks(3, range(1, 2))
            rope_m(3)
            attention(0, 2)
            qk_chunks(1, range(2))
            rope_m(1)
            attention(0, 3)
            for iq in range(4):
                attention(1, iq)
                for tn in range(4 * iq, 4 * iq + 4):
                    proj_tile(tn)
    nc.compile()
    _cache["nc"] = nc
    return nc


def kernel(x, w_qkv, w_out, b_out, _trace=False):
    import ml_dtypes
    from concourse.bass_utils import run_bass_kernel_spmd

    x = np.asarray(x, dtype=np.float32)
    w_qkv = np.asarray(w_qkv, dtype=np.float32)
    w_out = np.asarray(w_out, dtype=np.float32)
    b_out = np.asarray(b_out, dtype=np.float32)

    cos2, sin2 = _rope_tables()
    p2t = _p2t()

    in_maps = []
    for c in range(N_CORES):
        b, g = divmod(c, G)
        cols = []
        for blk in range(2):                      # q block, k block
            base = blk * H * DH + g * G * DH
            cols.append(w_qkv[:, base:base + G * DH])
        wqk_c = np.ascontiguousarray(np.concatenate(cols, axis=1))  # [DIM, 512]
        wv_c = np.ascontiguousarray(
            w_qkv[:, 2 * H * DH + g * G * DH: 2 * H * DH + (g + 1) * G * DH])
        wout_c = np.ascontiguousarray(
            w_out[g * G * DH:(g + 1) * G * DH, :]).astype(ml_dtypes.bfloat16)
        in_maps.append({
            "xT": np.ascontiguousarray(x[b].T).astype(ml_dtypes.bfloat16),
            "wqk": wqk_c.astype(ml_dtypes.bfloat16),
            "wv": wv_c.astype(ml_dtypes.bfloat16),
            "wout": wout_c,
            "cos2": cos2.astype(ml_dtypes.bfloat16),
            "sin2": sin2.astype(ml_dtypes.bfloat16),
            "p2t": p2t.astype(ml_dtypes.bfloat16),
        })

    nc = _build()
    res = run_bass_kernel_spmd(nc, in_maps, core_ids=list(range(N_CORES)),
                               trace=_trace)
    out = np.empty((B, N, DIM), dtype=np.float32)
    for b in range(B):
        acc = res.results[G * b]["part"].astype(np.float32)
        for g in range(1, G):
            acc += res.results[G * b + g]["part"].astype(np.float32)
        out[b] = acc + b_out
    if _trace:
        kernel.last_results = res
    return out


# revision 14
# speedup vs baseline: 1.0888x; 1.0888x over previous
"""Trainium2 Bass kernel for nn_Attention_35021163332119.

Full multi-head attention: qkv = x @ w_qkv; RoPE(q, k); softmax(q k^T / sqrt(dh)) v;
out = heads @ w_out + b_out.  B=2, N=2048, DIM=1024, H=16, DH=64.

Sharding: 8 cores = (batch b in {0,1}) x (head-group g in {0..3} of 4 heads).
Each core computes its 4 heads end-to-end plus the partial output projection
for its head-group's rows of w_out; the host sums the 4 partials per batch
(in fp32, from fp16 device partials) and adds b_out.

On-core layout: x is host-transposed to xT [DIM, N] so the contraction dim
sits on SBUF partitions.  q,k are produced transposed ([dh, n], head pairs
stacked on 128 partitions) straight out of the QKV matmul; v is produced in
natural [n, dh] layout with an extra ones column, so the PV matmul (M=65)
also accumulates the softmax denominator in row 64.  RoPE's interleaved
pair-rotation is a 128x128 +/-1 permutation matmul on the PE plus DVE
multiplies against cos/sin tables.

Everything on the probability/QK side runs in bf16 (validated 7e-3 rel err
vs the 2e-2 gate): bf16 weights enable fast weight load on the PE and 2x/4x
DVE modes for the RoPE elementwise work.  Input DMA is stripe-major (weights
first, then xT in 512-column stripes across all k-tiles) so the first QKV
matmul starts as soon as ~1.5 MB has landed.  The second head-pair's QKV
matmuls and the output projection are interleaved into the attention phase
to fill PE slack under the scalar-engine exp stream.  Partial outputs DMA
out as fp16.
"""

import numpy as np

B, N, DIM, H, DH = 2, 2048, 1024, 16, 64
ROPE_BASE = 10000.0
SCALE = DH ** -0.5
N_CORES = 8
G = 4                 # heads per core
KT = DIM // 128       # contraction tiles
NT = N // 128         # sequence tiles

_cache = {}


def _rope_tables():
    inv_freq = (1.0 / (ROPE_BASE ** (np.arange(0, DH, 2, dtype=np.float32) / DH)))
    t = np.arange(N, dtype=np.float32)
    freqs = t[:, None] * inv_freq[None, :]          # [N, DH/2]
    freqs = np.repeat(freqs, 2, axis=-1)            # [N, DH] interleaved
    cosT = np.cos(freqs).T.astype(np.float32)       # [DH, N]
    sinT = np.sin(freqs).T.astype(np.float32)
    cos2 = np.concatenate([cosT, cosT], axis=0)     # [128, N] two heads stacked
    sin2 = np.concatenate([sinT, sinT], axis=0)
    return np.ascontiguousarray(cos2), np.ascontiguousarray(sin2)


def _p2t():
    # rot = P2 @ qT with P2 = blockdiag(P, P), P[2t, 2t+1] = -1, P[2t+1, 2t] = 1
    # matmul computes lhsT.T @ rhs, so pass P2.T
    p = np.zeros((DH, DH), dtype=np.float32)
    for t in range(DH // 2):
        p[2 * t, 2 * t + 1] = -1.0
        p[2 * t + 1, 2 * t] = 1.0
    p2 = np.zeros((128, 128), dtype=np.float32)
    p2[:DH, :DH] = p
    p2[DH:, DH:] = p
    return np.ascontiguousarray(p2.T)


def _build():
    if "nc" in _cache:
        return _cache["nc"]

    import concourse.mybir as mybir
    import concourse.tile as tile
    from concourse import bacc

    F32 = mybir.dt.float32
    F16 = mybir.dt.float16
    BF16 = mybir.dt.bfloat16
    EXP = mybir.ActivationFunctionType.Exp

    nc = bacc.Bacc("TRN2", target_bir_lowering=False, debug=False)
    xT_d = nc.dram_tensor("xT", [DIM, N], BF16, kind="ExternalInput")
    wqk_d = nc.dram_tensor("wqk", [DIM, 4 * 128], BF16, kind="ExternalInput")
    wv_d = nc.dram_tensor("wv", [DIM, G * DH], BF16, kind="ExternalInput")
    wout_d = nc.dram_tensor("wout", [G * DH, DIM], BF16, kind="ExternalInput")
    cos_d = nc.dram_tensor("cos2", [128, N], BF16, kind="ExternalInput")
    sin_d = nc.dram_tensor("sin2", [128, N], BF16, kind="ExternalInput")
    p2t_d = nc.dram_tensor("p2t", [128, 128], BF16, kind="ExternalInput")
    part_d = nc.dram_tensor("part", [N, DIM], F16, kind="ExternalOutput")

    with tile.TileContext(nc) as tc:
        with tc.tile_pool(name="persist", bufs=1) as persist, \
             tc.tile_pool(name="att", bufs=8) as att, \
             tc.tile_pool(name="norm_w", bufs=2) as norm_w, \
             tc.tile_pool(name="outp", bufs=3) as outp, \
             tc.tile_pool(name="xph", bufs=1) as xph, \
             tc.tile_pool(name="rope_w", bufs=2) as rope_w, \
             tc.tile_pool(name="ps", bufs=3, space="PSUM") as ps, \
             tc.tile_pool(name="pso", bufs=2, space="PSUM") as pso:

            # ---- persistent tiles ----
            qk_sb = [persist.tile([128, N], BF16, tag=f"qk{m}", name=f"qk{m}")
                     for m in range(4)]          # q01T, q23T, k01T, k23T
            v_aug = persist.tile([128, NT, G, DH + 1], BF16, tag="vaug")
            wout_sb = [persist.tile([128, DIM], BF16, tag=f"wo{kk}", name=f"wo{kk}")
                       for kk in range(2)]
            outT = [persist.tile([128, N], BF16, tag=f"outT{p}", name=f"outT{p}")
                    for p in range(2)]

            # ---- phase-1 tiles ----
            xT = [xph.tile([128, N], BF16, tag=f"xT{k}", name=f"xT{k}")
                  for k in range(KT)]
            wqk = [xph.tile([128, 4 * 128], BF16, tag=f"wqk{k}", name=f"wqk{k}")
                   for k in range(KT)]
            wv = [xph.tile([128, G * DH], BF16, tag=f"wv{k}", name=f"wv{k}")
                  for k in range(KT)]
            cos2 = xph.tile([128, N], BF16, tag="cos2")
            sin2 = xph.tile([128, N], BF16, tag="sin2")
            p2t = xph.tile([128, 128], BF16, tag="p2t")
            ones_col = xph.tile([128, NT, G, 1], F32, tag="ones")

            # qk weights first, then xT stripe-major; tables and the
            # projection weights slot in behind the first stripe (they are
            # first needed at rope / projection time)
            for k in range(KT):
                nc.sync.dma_start(
                    out=wqk[k],
                    in_=wqk_d.ap().rearrange("(t p) m -> t p m", p=128)[k])
            nc.vector.memset(ones_col, 1.0)
            for c in range(4):
                csl = slice(c * 512, (c + 1) * 512)
                for k in range(KT):
                    nc.sync.dma_start(
                        out=xT[k][:, csl],
                        in_=xT_d.ap().rearrange(
                            "(t p) n -> t p n", p=128)[k][:, csl])
                if c == 0:
                    nc.sync.dma_start(out=p2t, in_=p2t_d.ap())
                    nc.sync.dma_start(out=sin2, in_=sin_d.ap())
                    nc.sync.dma_start(out=cos2, in_=cos_d.ap())
                elif c == 1:
                    for k in range(KT):
                        nc.sync.dma_start(
                            out=wv[k],
                            in_=wv_d.ap().rearrange(
                                "(t p) m -> t p m", p=128)[k])
                elif c == 2:
                    for kk in range(2):
                        nc.sync.dma_start(
                            out=wout_sb[kk],
                            in_=wout_d.ap().rearrange(
                                "(t p) m -> t p m", p=128)[kk])

            def qk_chunks(m, crange):
                # m: 0,1 = q01T,q23T; 2,3 = k01T,k23T.  crange indexes 1024-
                # wide chunks (two 512 halves per PSUM tile).  pair-0 evacs on
                # the scalar engine (idle during lead-in); pair-1 evacs on DVE
                # so the scalar engine stays pure-exp during attention.
                for c2 in crange:
                    mm = ps.tile([128, 1024], F32, tag="s", name="mm_qk")
                    for half in range(2):
                        hsl = slice(half * 512, (half + 1) * 512)
                        csl = slice(c2 * 1024 + half * 512,
                                    c2 * 1024 + (half + 1) * 512)
                        for k in range(KT):
                            nc.tensor.matmul(
                                mm[:, hsl],
                                wqk[k][:, m * 128:(m + 1) * 128],
                                xT[k][:, csl],
                                start=(k == 0), stop=(k == KT - 1))
                    osl = slice(c2 * 1024, (c2 + 1) * 1024)
                    if m in (0, 2):
                        nc.scalar.copy(qk_sb[m][:, osl], mm)
                    else:
                        nc.vector.tensor_copy(qk_sb[m][:, osl], mm)

            def rope_m(m):
                tmp = rope_w.tile([128, N], BF16, tag="ropetmp")
                for c2 in range(2):
                    rot = ps.tile([128, 1024], F32, tag="s", name="mm_rot")
                    for half in range(2):
                        csl = slice(c2 * 1024 + half * 512,
                                    c2 * 1024 + (half + 1) * 512)
                        nc.tensor.matmul(
                            rot[:, half * 512:(half + 1) * 512],
                            p2t, qk_sb[m][:, csl],
                            start=True, stop=True)
                    osl = slice(c2 * 1024, (c2 + 1) * 1024)
                    nc.vector.tensor_mul(tmp[:, osl], rot, sin2[:, osl])
                nc.vector.tensor_mul(qk_sb[m], qk_sb[m], cos2)
                nc.vector.tensor_add(qk_sb[m], qk_sb[m], tmp)

            def v_tiles(trange):
                for th in trange:           # one psum tile covers 4 n-tiles
                    mm = ps.tile([128, 1024], F32, tag="s", name="mm_v")
                    for quad in range(4):
                        tn = 4 * th + quad
                        for k in range(KT):
                            nc.tensor.matmul(
                                mm[:, quad * 256:(quad + 1) * 256],
                                xT[k][:, tn * 128:(tn + 1) * 128],
                                wv[k],
                                start=(k == 0), stop=(k == KT - 1))
                    nc.scalar.copy(
                        v_aug[:, 4 * th:4 * th + 4, :, 0:DH],
                        mm.rearrange("p (t h d) -> p t h d", t=4, h=G))
                if trange and trange[-1] == NT // 4 - 1:
                    nc.gpsimd.tensor_copy(v_aug[:, :, :, DH:DH + 1], ones_col)

            def attention(p, iq):
                """One (head-pair, i-quarter of 512) block.  exp of jj runs on
                the scalar engine while the PE does PV of jj-1."""
                qT = qk_sb[p]
                kTt = qk_sb[2 + p]
                i0 = iq * 512
                isl = slice(i0, i0 + 512)
                o_ps = [pso.tile([DH + 1, 512], F32, tag="o", name=f"o{hh}")
                        for hh in range(2)]

                def emit_pv(jj, exps):
                    for hh in range(2):
                        for half in range(2):
                            j = 2 * jj + half
                            nc.tensor.matmul(
                                o_ps[hh],
                                v_aug[:, j, 2 * p + hh, :],
                                exps[hh][:, half * 512:(half + 1) * 512],
                                start=(j == 0), stop=(j == NT - 1))

                pend = None
                for jj in range(NT // 2):
                    s_ps = [ps.tile([128, 1024], F32, tag="s", name=f"s{hh}")
                            for hh in range(2)]
                    for half in range(2):
                        j = 2 * jj + half
                        jsl = slice(j * 128, (j + 1) * 128)
                        for hh in range(2):
                            hsl = slice(hh * DH, (hh + 1) * DH)
                            nc.tensor.matmul(
                                s_ps[hh][:, half * 512:(half + 1) * 512],
                                kTt[hsl, jsl], qT[hsl, isl],
                                start=True, stop=True)
                    exps = []
                    for hh in range(2):
                        expT = att.tile([128, 1024], BF16, tag="exp")
                        nc.scalar.activation(expT, s_ps[hh], EXP, scale=SCALE)
                        exps.append(expT)
                    if pend is not None:
                        emit_pv(jj - 1, pend)
                    pend = exps
                emit_pv(NT // 2 - 1, pend)
                return o_ps

            def att_norm(p, iq, o_ps):
                # evacuate PV accumulators so PSUM frees fast, then normalize
                # off the critical path
                isl = slice(iq * 512, (iq + 1) * 512)
                for hh in range(2):
                    o_sb = norm_w.tile([DH + 1, 512], F32, tag=f"osb{hh}",
                                       name=f"osb{hh}")
                    nc.vector.tensor_copy(o_sb, o_ps[hh])
                    recip0 = norm_w.tile([1, 512], F32, tag=f"r0{hh}",
                                         name=f"r0{hh}")
                    nc.sync.dma_start(out=recip0, in_=o_sb[DH:DH + 1, :])
                    nc.vector.reciprocal_approx_fast(recip0, recip0)
                    bc = norm_w.tile([DH, 512], F32, tag=f"bc{hh}",
                                     name=f"bc{hh}")
                    nc.gpsimd.partition_broadcast(bc, recip0)
                    if hh == 0:
                        nc.vector.tensor_mul(outT[p][0:DH, isl],
                                             o_sb[0:DH, :], bc)
                    else:
                        tmpb = norm_w.tile([DH, 512], BF16, tag="tmpb")
                        nc.vector.tensor_mul(tmpb, o_sb[0:DH, :], bc)
                        nc.sync.dma_start(out=outT[p][DH:2 * DH, isl],
                                          in_=tmpb)

            def proj_tile(tn):
                nsl = slice(tn * 128, (tn + 1) * 128)
                f_ps = ps.tile([128, 1024], F32, tag="s", name="f_ps")
                for c2 in range(2):
                    c2sl = slice(c2 * 512, (c2 + 1) * 512)
                    for kk in range(2):
                        nc.tensor.matmul(
                            f_ps[:, c2sl],
                            outT[kk][:, nsl], wout_sb[kk][:, c2sl],
                            start=(kk == 0), stop=(kk == 1))
                out_sb = outp.tile([128, DIM], F16, tag="osb")
                nc.vector.tensor_copy(out_sb, f_ps)
                nc.sync.dma_start(
                    out=part_d.ap().rearrange("(t p) m -> t p m", p=128)[tn],
                    in_=out_sb)

            # ---- emission order ----
            # lead-in: k01 then q01 (pair 0) + v; pair 1 QKV and the output
            # projection interleave into the attention phase.
            def att_block(p, iq):
                att_norm(p, iq, attention(p, iq))

            qk_chunks(2, range(2))
            rope_m(2)
            qk_chunks(0, range(2))
            rope_m(0)
            v_tiles(range(NT // 4))
            att_block(0, 0)
            qk_chunks(3, range(1))
            att_block(0, 1)
            qk_chunks(3, range(1, 2))
            rope_m(3)
            att_block(0, 2)
            qk_chunks(1, range(2))
            rope_m(1)
            att_block(0, 3)
            # proj for i-block iq trails by one attention block so its PE
            # matmuls never wait on the just-emitted norm chain
            att_block(1, 0)
            att_block(1, 1)
            for tn in range(0, 4):
                proj_tile(tn)
            att_block(1, 2)
            for tn in range(4, 8):
                proj_tile(tn)
            o_last = attention(1, 3)
            for tn in range(8, 12):
                proj_tile(tn)
            att_norm(1, 3, o_last)
            for tn in range(12, 16):
                proj_tile(tn)
    nc.compile()
    _cache["nc"] = nc
    return nc


def kernel(x, w_qkv, w_out, b_out, _trace=False):
    import ml_dtypes
    from concourse.bass_utils import run_bass_kernel_spmd

    x = np.asarray(x, dtype=np.float32)
    w_qkv = np.asarray(w_qkv, dtype=np.float32)
    w_out = np.asarray(w_out, dtype=np.float32)
    b_out = np.asarray(b_out, dtype=np.float32)

    cos2, sin2 = _rope_tables()
    p2t = _p2t()

    in_maps = []
    for c in range(N_CORES):
        b, g = divmod(c, G)
        cols = []
        for blk in range(2):                      # q block, k block
            base = blk * H * DH + g * G * DH
            cols.append(w_qkv[:, base:base + G * DH])
        wqk_c = np.ascontiguousarray(np.concatenate(cols, axis=1))  # [DIM, 512]
        wv_c = np.ascontiguousarray(
            w_qkv[:, 2 * H * DH + g * G * DH: 2 * H * DH + (g + 1) * G * DH])
        wout_c = np.ascontiguousarray(
            w_out[g * G * DH:(g + 1) * G * DH, :]).astype(ml_dtypes.bfloat16)
        in_maps.append({
            "xT": np.ascontiguousarray(x[b].T).astype(ml_dtypes.bfloat16),
            "wqk": wqk_c.astype(ml_dtypes.bfloat16),
            "wv": wv_c.astype(ml_dtypes.bfloat16),
            "wout": wout_c,
            "cos2": cos2.astype(ml_dtypes.bfloat16),
            "sin2": sin2.astype(ml_dtypes.bfloat16),
            "p2t": p2t.astype(ml_dtypes.bfloat16),
        })

    nc = _build()
    res = run_bass_kernel_spmd(nc, in_maps, core_ids=list(range(N_CORES)),
                               trace=_trace)
    out = np.empty((B, N, DIM), dtype=np.float32)
    for b in range(B):
        acc = res.results[G * b]["part"].astype(np.float32)
        for g in range(1, G):
            acc += res.results[G * b + g]["part"].astype(np.float32)
        out[b] = acc + b_out
    if _trace:
        kernel.last_results = res
    return out


# revision 19
# speedup vs baseline: 1.1235x; 1.0319x over previous
"""Trainium2 Bass kernel for nn_Attention_35021163332119.

Full multi-head attention: qkv = x @ w_qkv; RoPE(q, k); softmax(q k^T / sqrt(dh)) v;
out = heads @ w_out + b_out.  B=2, N=2048, DIM=1024, H=16, DH=64.

Sharding: 8 cores = (batch b in {0,1}) x (head-group g in {0..3} of 4 heads).
Each core computes its 4 heads end-to-end plus the partial output projection
for its head-group's rows of w_out; the host sums the 4 partials per batch
(in fp32, from fp16 device partials) and adds b_out.

On-core layout: x is host-transposed to xT [DIM, N] so the contraction dim
sits on SBUF partitions.  q,k are produced transposed ([dh, n], head pairs
stacked on 128 partitions) straight out of the QKV matmul; v is produced in
natural [n, dh] layout with an extra ones column, so the PV matmul (M=65)
also accumulates the softmax denominator in row 64.  RoPE's interleaved
pair-rotation is a 128x128 +/-1 permutation matmul on the PE plus DVE
multiplies against cos/sin tables.

Everything on the probability/QK side runs in bf16 (validated 7e-3 rel err
vs the 2e-2 gate): bf16 weights enable fast weight load on the PE and 2x/4x
DVE modes for the RoPE elementwise work.  Input DMA is stripe-major (weights
first, then xT in 512-column stripes across all k-tiles) so the first QKV
matmul starts as soon as ~1.5 MB has landed.  The second head-pair's QKV
matmuls and the output projection are interleaved into the attention phase
to fill PE slack under the scalar-engine exp stream.  Partial outputs DMA
out as fp16.
"""

import numpy as np

B, N, DIM, H, DH = 2, 2048, 1024, 16, 64
ROPE_BASE = 10000.0
SCALE = DH ** -0.5
N_CORES = 8
G = 4                 # heads per core
KT = DIM // 128       # contraction tiles
NT = N // 128         # sequence tiles

_cache = {}


def _rope_tables():
    inv_freq = (1.0 / (ROPE_BASE ** (np.arange(0, DH, 2, dtype=np.float32) / DH)))
    t = np.arange(N, dtype=np.float32)
    freqs = t[:, None] * inv_freq[None, :]          # [N, DH/2]
    freqs = np.repeat(freqs, 2, axis=-1)            # [N, DH] interleaved
    cosT = np.cos(freqs).T.astype(np.float32)       # [DH, N]
    sinT = np.sin(freqs).T.astype(np.float32)
    cos2 = np.concatenate([cosT, cosT], axis=0)     # [128, N] two heads stacked
    sin2 = np.concatenate([sinT, sinT], axis=0)
    return np.ascontiguousarray(cos2), np.ascontiguousarray(sin2)


def _p2t():
    # rot = P2 @ qT with P2 = blockdiag(P, P), P[2t, 2t+1] = -1, P[2t+1, 2t] = 1
    # matmul computes lhsT.T @ rhs, so pass P2.T
    p = np.zeros((DH, DH), dtype=np.float32)
    for t in range(DH // 2):
        p[2 * t, 2 * t + 1] = -1.0
        p[2 * t + 1, 2 * t] = 1.0
    p2 = np.zeros((128, 128), dtype=np.float32)
    p2[:DH, :DH] = p
    p2[DH:, DH:] = p
    return np.ascontiguousarray(p2.T)


def _build():
    if "nc" in _cache:
        return _cache["nc"]

    import concourse.mybir as mybir
    import concourse.tile as tile
    from concourse import bacc

    F32 = mybir.dt.float32
    F16 = mybir.dt.float16
    BF16 = mybir.dt.bfloat16
    EXP = mybir.ActivationFunctionType.Exp

    nc = bacc.Bacc("TRN2", target_bir_lowering=False, debug=False)
    xT_d = nc.dram_tensor("xT", [DIM, N], BF16, kind="ExternalInput")
    wqk_d = nc.dram_tensor("wqk", [DIM, 4 * 128], BF16, kind="ExternalInput")
    wv_d = nc.dram_tensor("wv", [DIM, G * DH], BF16, kind="ExternalInput")
    wout_d = nc.dram_tensor("wout", [G * DH, DIM], BF16, kind="ExternalInput")
    cos_d = nc.dram_tensor("cos2", [128, N], BF16, kind="ExternalInput")
    sin_d = nc.dram_tensor("sin2", [128, N], BF16, kind="ExternalInput")
    p2t_d = nc.dram_tensor("p2t", [128, 128], BF16, kind="ExternalInput")
    part_d = nc.dram_tensor("part", [N, DIM], F16, kind="ExternalOutput")

    with tile.TileContext(nc) as tc:
        with tc.tile_pool(name="persist", bufs=1) as persist, \
             tc.tile_pool(name="att", bufs=8) as att, \
             tc.tile_pool(name="norm_w", bufs=2) as norm_w, \
             tc.tile_pool(name="outp", bufs=3) as outp, \
             tc.tile_pool(name="xph", bufs=1) as xph, \
             tc.tile_pool(name="rope_w", bufs=2) as rope_w, \
             tc.tile_pool(name="ps", bufs=3, space="PSUM") as ps, \
             tc.tile_pool(name="pso", bufs=2, space="PSUM") as pso:

            # ---- persistent tiles ----
            qk_sb = [persist.tile([128, N], BF16, tag=f"qk{m}", name=f"qk{m}")
                     for m in range(4)]          # q01T, q23T, k01T, k23T
            v_aug = persist.tile([128, NT, G, DH + 1], BF16, tag="vaug")
            wout_sb = [persist.tile([128, DIM], BF16, tag=f"wo{kk}", name=f"wo{kk}")
                       for kk in range(2)]
            outT = [persist.tile([128, N], BF16, tag=f"outT{p}", name=f"outT{p}")
                    for p in range(2)]

            # ---- phase-1 tiles ----
            xT = [xph.tile([128, N], BF16, tag=f"xT{k}", name=f"xT{k}")
                  for k in range(KT)]
            wqk = [xph.tile([128, 4 * 128], BF16, tag=f"wqk{k}", name=f"wqk{k}")
                   for k in range(KT)]
            wv = [xph.tile([128, G * DH], BF16, tag=f"wv{k}", name=f"wv{k}")
                  for k in range(KT)]
            cos2 = xph.tile([128, N], BF16, tag="cos2")
            sin2 = xph.tile([128, N], BF16, tag="sin2")
            p2t = xph.tile([128, 128], BF16, tag="p2t")
            ones_col = xph.tile([128, NT, G, 1], F32, tag="ones")

            # qk weights first, then xT stripe-major; tables and the
            # projection weights slot in behind the first stripe (they are
            # first needed at rope / projection time)
            for k in range(KT):
                nc.sync.dma_start(
                    out=wqk[k],
                    in_=wqk_d.ap().rearrange("(t p) m -> t p m", p=128)[k])
            nc.vector.memset(ones_col, 1.0)
            for c in range(4):
                csl = slice(c * 512, (c + 1) * 512)
                for k in range(KT):
                    nc.sync.dma_start(
                        out=xT[k][:, csl],
                        in_=xT_d.ap().rearrange(
                            "(t p) n -> t p n", p=128)[k][:, csl])
                if c == 0:
                    nc.sync.dma_start(out=p2t, in_=p2t_d.ap())
                    nc.sync.dma_start(out=sin2, in_=sin_d.ap())
                    nc.sync.dma_start(out=cos2, in_=cos_d.ap())
                elif c == 1:
                    for k in range(KT):
                        nc.sync.dma_start(
                            out=wv[k],
                            in_=wv_d.ap().rearrange(
                                "(t p) m -> t p m", p=128)[k])
                elif c == 2:
                    for kk in range(2):
                        nc.sync.dma_start(
                            out=wout_sb[kk],
                            in_=wout_d.ap().rearrange(
                                "(t p) m -> t p m", p=128)[kk])

            def qk_chunks(m, crange):
                # m: 0,1 = q01T,q23T; 2,3 = k01T,k23T.  crange indexes 1024-
                # wide chunks (two 512 halves per PSUM tile).  pair-0 evacs on
                # the scalar engine (idle during lead-in); pair-1 evacs on DVE
                # so the scalar engine stays pure-exp during attention.
                for c2 in crange:
                    mm = ps.tile([128, 1024], F32, tag="s", name="mm_qk")
                    for half in range(2):
                        hsl = slice(half * 512, (half + 1) * 512)
                        csl = slice(c2 * 1024 + half * 512,
                                    c2 * 1024 + (half + 1) * 512)
                        for k in range(KT):
                            nc.tensor.matmul(
                                mm[:, hsl],
                                wqk[k][:, m * 128:(m + 1) * 128],
                                xT[k][:, csl],
                                start=(k == 0), stop=(k == KT - 1))
                    osl = slice(c2 * 1024, (c2 + 1) * 1024)
                    if m in (0, 2):
                        nc.scalar.copy(qk_sb[m][:, osl], mm)
                    else:
                        nc.vector.tensor_copy(qk_sb[m][:, osl], mm)

            def rope_m(m):
                tmp = rope_w.tile([128, N], BF16, tag="ropetmp")
                for c2 in range(2):
                    rot = ps.tile([128, 1024], F32, tag="s", name="mm_rot")
                    for half in range(2):
                        csl = slice(c2 * 1024 + half * 512,
                                    c2 * 1024 + (half + 1) * 512)
                        nc.tensor.matmul(
                            rot[:, half * 512:(half + 1) * 512],
                            p2t, qk_sb[m][:, csl],
                            start=True, stop=True)
                    osl = slice(c2 * 1024, (c2 + 1) * 1024)
                    nc.vector.tensor_mul(tmp[:, osl], rot, sin2[:, osl])
                nc.vector.tensor_mul(qk_sb[m], qk_sb[m], cos2)
                nc.vector.tensor_add(qk_sb[m], qk_sb[m], tmp)

            def v_tiles(trange):
                for th in trange:           # one psum tile covers 4 n-tiles
                    mm = ps.tile([128, 1024], F32, tag="s", name="mm_v")
                    for quad in range(4):
                        tn = 4 * th + quad
                        for k in range(KT):
                            nc.tensor.matmul(
                                mm[:, quad * 256:(quad + 1) * 256],
                                xT[k][:, tn * 128:(tn + 1) * 128],
                                wv[k],
                                start=(k == 0), stop=(k == KT - 1))
                    nc.vector.tensor_copy(
                        v_aug[:, 4 * th:4 * th + 4, :, 0:DH],
                        mm.rearrange("p (t h d) -> p t h d", t=4, h=G))
                if trange and trange[-1] == NT // 4 - 1:
                    nc.gpsimd.tensor_copy(v_aug[:, :, :, DH:DH + 1], ones_col)

            def attention(p, iq, filler=None):
                """One (head-pair, i-quarter of 512) block.  exp of jj runs on
                the scalar engine while the PE does PV of jj-1.  `filler`
                emits independent PE work between the last exps and the final
                PV group, covering the PE stall while the last exp drains."""
                qT = qk_sb[p]
                kTt = qk_sb[2 + p]
                i0 = iq * 512
                isl = slice(i0, i0 + 512)
                o_ps = [pso.tile([DH + 1, 512], F32, tag="o", name=f"o{hh}")
                        for hh in range(2)]

                def emit_pv(jj, exps):
                    for hh in range(2):
                        for half in range(2):
                            j = 2 * jj + half
                            nc.tensor.matmul(
                                o_ps[hh],
                                v_aug[:, j, 2 * p + hh, :],
                                exps[hh][:, half * 512:(half + 1) * 512],
                                start=(j == 0), stop=(j == NT - 1))

                pend = None
                for jj in range(NT // 2):
                    s_ps = [ps.tile([128, 1024], F32, tag="s", name=f"s{hh}")
                            for hh in range(2)]
                    for half in range(2):
                        j = 2 * jj + half
                        jsl = slice(j * 128, (j + 1) * 128)
                        for hh in range(2):
                            hsl = slice(hh * DH, (hh + 1) * DH)
                            nc.tensor.matmul(
                                s_ps[hh][:, half * 512:(half + 1) * 512],
                                kTt[hsl, jsl], qT[hsl, isl],
                                start=True, stop=True)
                    exps = []
                    for hh in range(2):
                        expT = att.tile([128, 1024], BF16, tag="exp")
                        nc.scalar.activation(expT, s_ps[hh], EXP, scale=SCALE)
                        exps.append(expT)
                    if pend is not None:
                        emit_pv(jj - 1, pend)
                    pend = exps
                if filler is not None:
                    filler()
                emit_pv(NT // 2 - 1, pend)
                return o_ps

            def att_norm(p, iq, o_ps):
                # evacuate PV accumulators so PSUM frees fast, then normalize
                # off the critical path
                isl = slice(iq * 512, (iq + 1) * 512)
                for hh in range(2):
                    o_sb = norm_w.tile([DH + 1, 512], F32, tag=f"osb{hh}",
                                       name=f"osb{hh}")
                    nc.vector.tensor_copy(o_sb, o_ps[hh])
                    recip0 = norm_w.tile([1, 512], F32, tag=f"r0{hh}",
                                         name=f"r0{hh}")
                    nc.sync.dma_start(out=recip0, in_=o_sb[DH:DH + 1, :])
                    nc.vector.reciprocal_approx_fast(recip0, recip0)
                    bc = norm_w.tile([DH, 512], F32, tag=f"bc{hh}",
                                     name=f"bc{hh}")
                    nc.gpsimd.partition_broadcast(bc, recip0)
                    if hh == 0:
                        nc.vector.tensor_mul(outT[p][0:DH, isl],
                                             o_sb[0:DH, :], bc)
                    else:
                        tmpb = norm_w.tile([DH, 512], BF16, tag="tmpb")
                        nc.vector.tensor_mul(tmpb, o_sb[0:DH, :], bc)
                        nc.sync.dma_start(out=outT[p][DH:2 * DH, isl],
                                          in_=tmpb)

            def proj_tile(tn):
                nsl = slice(tn * 128, (tn + 1) * 128)
                f_ps = ps.tile([128, 1024], F32, tag="s", name="f_ps")
                for c2 in range(2):
                    c2sl = slice(c2 * 512, (c2 + 1) * 512)
                    for kk in range(2):
                        nc.tensor.matmul(
                            f_ps[:, c2sl],
                            outT[kk][:, nsl], wout_sb[kk][:, c2sl],
                            start=(kk == 0), stop=(kk == 1))
                out_sb = outp.tile([128, DIM], F16, tag="osb")
                nc.vector.tensor_copy(out_sb, f_ps)
                nc.sync.dma_start(
                    out=part_d.ap().rearrange("(t p) m -> t p m", p=128)[tn],
                    in_=out_sb)

            # ---- emission order ----
            # lead-in: k01 then q01 (pair 0) + v; pair 1 QKV and the output
            # projection interleave into the attention phase.
            def att_block(p, iq, filler=None):
                att_norm(p, iq, attention(p, iq, filler))

            def projs(*tns):
                def f():
                    for tn in tns:
                        proj_tile(tn)
                return f

            qk_chunks(2, range(2))
            rope_m(2)
            qk_chunks(0, range(2))
            rope_m(0)
            v_tiles(range(NT // 4))
            att_block(0, 0, lambda: qk_chunks(3, range(1)))
            att_block(0, 1, lambda: qk_chunks(3, range(1, 2)))
            rope_m(3)
            att_block(0, 2, lambda: qk_chunks(1, range(2)))
            rope_m(1)
            att_block(0, 3)
            # proj for i-block iq trails by one attention block so its PE
            # matmuls never wait on the just-emitted norm chain
            att_block(1, 0)
            att_block(1, 1, projs(0, 1))
            projs(2, 3)()
            att_block(1, 2, projs(4, 5))
            projs(6, 7)()
            o_last = attention(1, 3, projs(8, 9))
            att_norm(1, 3, o_last)
            projs(10, 11)()
            for tn in range(12, 16):
                proj_tile(tn)
    nc.compile()
    _cache["nc"] = nc
    return nc


def kernel(x, w_qkv, w_out, b_out, _trace=False):
    import ml_dtypes
    from concourse.bass_utils import run_bass_kernel_spmd

    x = np.asarray(x, dtype=np.float32)
    w_qkv = np.asarray(w_qkv, dtype=np.float32)
    w_out = np.asarray(w_out, dtype=np.float32)
    b_out = np.asarray(b_out, dtype=np.float32)

    cos2, sin2 = _rope_tables()
    p2t = _p2t()

    in_maps = []
    for c in range(N_CORES):
        b, g = divmod(c, G)
        cols = []
        for blk in range(2):                      # q block, k block
            base = blk * H * DH + g * G * DH
            cols.append(w_qkv[:, base:base + G * DH])
        wqk_c = np.ascontiguousarray(np.concatenate(cols, axis=1))  # [DIM, 512]
        wv_c = np.ascontiguousarray(
            w_qkv[:, 2 * H * DH + g * G * DH: 2 * H * DH + (g + 1) * G * DH])
        wout_c = np.ascontiguousarray(
            w_out[g * G * DH:(g + 1) * G * DH, :]).astype(ml_dtypes.bfloat16)
        in_maps.append({
            "xT": np.ascontiguousarray(x[b].T).astype(ml_dtypes.bfloat16),
            "wqk": wqk_c.astype(ml_dtypes.bfloat16),
            "wv": wv_c.astype(ml_dtypes.bfloat16),
            "wout": wout_c,
            "cos2": cos2.astype(ml_dtypes.bfloat16),
            "sin2": sin2.astype(ml_dtypes.bfloat16),
            "p2t": p2t.astype(ml_dtypes.bfloat16),
        })

    nc = _build()
    res = run_bass_kernel_spmd(nc, in_maps, core_ids=list(range(N_CORES)),
                               trace=_trace)
    out = np.empty((B, N, DIM), dtype=np.float32)
    for b in range(B):
        acc = res.results[G * b]["part"].astype(np.float32)
        for g in range(1, G):
            acc += res.results[G * b + g]["part"].astype(np.float32)
        out[b] = acc + b_out
    if _trace:
        kernel.last_results = res
    return out


# revision 21
# speedup vs baseline: 1.1551x; 1.0281x over previous
"""Trainium2 Bass kernel for nn_Attention_35021163332119.

Full multi-head attention: qkv = x @ w_qkv; RoPE(q, k); softmax(q k^T / sqrt(dh)) v;
out = heads @ w_out + b_out.  B=2, N=2048, DIM=1024, H=16, DH=64.

Sharding: 8 cores = (batch b in {0,1}) x (head-group g in {0..3} of 4 heads).
Each core computes its 4 heads end-to-end plus the partial output projection
for its head-group's rows of w_out; the host sums the 4 partials per batch
(in fp32, from fp16 device partials) and adds b_out.

On-core layout: x is host-transposed to xT [DIM, N] so the contraction dim
sits on SBUF partitions.  q,k are produced transposed ([dh, n], head pairs
stacked on 128 partitions) straight out of the QKV matmul; v is produced in
natural [n, dh] layout with an extra ones column, so the PV matmul (M=65)
also accumulates the softmax denominator in row 64.  RoPE's interleaved
pair-rotation is a 128x128 +/-1 permutation matmul on the PE plus DVE
multiplies against cos/sin tables.

Everything on the probability/QK side runs in bf16 (validated 7e-3 rel err
vs the 2e-2 gate): bf16 weights enable fast weight load on the PE and 2x/4x
DVE modes for the RoPE elementwise work.  Input DMA is stripe-major (weights
first, then xT in 512-column stripes across all k-tiles) so the first QKV
matmul starts as soon as ~1.5 MB has landed.  The second head-pair's QKV
matmuls and the output projection are interleaved into the attention phase
to fill PE slack under the scalar-engine exp stream.  Partial outputs DMA
out as fp16.
"""

import numpy as np

B, N, DIM, H, DH = 2, 2048, 1024, 16, 64
ROPE_BASE = 10000.0
SCALE = DH ** -0.5
N_CORES = 8
G = 4                 # heads per core
KT = DIM // 128       # contraction tiles
NT = N // 128         # sequence tiles

_cache = {}


def _rope_tables():
    inv_freq = (1.0 / (ROPE_BASE ** (np.arange(0, DH, 2, dtype=np.float32) / DH)))
    t = np.arange(N, dtype=np.float32)
    freqs = t[:, None] * inv_freq[None, :]          # [N, DH/2]
    freqs = np.repeat(freqs, 2, axis=-1)            # [N, DH] interleaved
    cosT = np.cos(freqs).T.astype(np.float32)       # [DH, N]
    sinT = np.sin(freqs).T.astype(np.float32)
    cos2 = np.concatenate([cosT, cosT], axis=0)     # [128, N] two heads stacked
    sin2 = np.concatenate([sinT, sinT], axis=0)
    return np.ascontiguousarray(cos2), np.ascontiguousarray(sin2)


def _p2t():
    # rot = P2 @ qT with P2 = blockdiag(P, P), P[2t, 2t+1] = -1, P[2t+1, 2t] = 1
    # matmul computes lhsT.T @ rhs, so pass P2.T
    p = np.zeros((DH, DH), dtype=np.float32)
    for t in range(DH // 2):
        p[2 * t, 2 * t + 1] = -1.0
        p[2 * t + 1, 2 * t] = 1.0
    p2 = np.zeros((128, 128), dtype=np.float32)
    p2[:DH, :DH] = p
    p2[DH:, DH:] = p
    return np.ascontiguousarray(p2.T)


def _build():
    if "nc" in _cache:
        return _cache["nc"]

    import concourse.mybir as mybir
    import concourse.tile as tile
    from concourse import bacc

    F32 = mybir.dt.float32
    F16 = mybir.dt.float16
    BF16 = mybir.dt.bfloat16
    EXP = mybir.ActivationFunctionType.Exp

    nc = bacc.Bacc("TRN2", target_bir_lowering=False, debug=False)
    xT_d = nc.dram_tensor("xT", [DIM, N], BF16, kind="ExternalInput")
    wqk_d = nc.dram_tensor("wqk", [DIM, 4 * 128], BF16, kind="ExternalInput")
    wv_d = nc.dram_tensor("wv", [DIM, G * DH], BF16, kind="ExternalInput")
    wout_d = nc.dram_tensor("wout", [G * DH, DIM], BF16, kind="ExternalInput")
    cos_d = nc.dram_tensor("cos2", [128, N], BF16, kind="ExternalInput")
    sin_d = nc.dram_tensor("sin2", [128, N], BF16, kind="ExternalInput")
    p2t_d = nc.dram_tensor("p2t", [128, 128], BF16, kind="ExternalInput")
    part_d = nc.dram_tensor("part", [N, DIM], F16, kind="ExternalOutput")

    with tile.TileContext(nc) as tc:
        with tc.tile_pool(name="persist", bufs=1) as persist, \
             tc.tile_pool(name="att", bufs=8) as att, \
             tc.tile_pool(name="norm_w", bufs=2) as norm_w, \
             tc.tile_pool(name="outp", bufs=3) as outp, \
             tc.tile_pool(name="xph", bufs=1) as xph, \
             tc.tile_pool(name="rope_w", bufs=2) as rope_w, \
             tc.tile_pool(name="ps", bufs=3, space="PSUM") as ps, \
             tc.tile_pool(name="pso", bufs=2, space="PSUM") as pso:

            # ---- persistent tiles ----
            qk_sb = [persist.tile([128, N], BF16, tag=f"qk{m}", name=f"qk{m}")
                     for m in range(4)]          # q01T, q23T, k01T, k23T
            v_aug = persist.tile([128, NT, G, DH + 1], BF16, tag="vaug")
            wout_sb = [persist.tile([128, DIM], BF16, tag=f"wo{kk}", name=f"wo{kk}")
                       for kk in range(2)]
            outT = [persist.tile([128, N], BF16, tag=f"outT{p}", name=f"outT{p}")
                    for p in range(2)]

            # ---- phase-1 tiles ----
            xT = [xph.tile([128, N], BF16, tag=f"xT{k}", name=f"xT{k}")
                  for k in range(KT)]
            wqk = [xph.tile([128, 4 * 128], BF16, tag=f"wqk{k}", name=f"wqk{k}")
                   for k in range(KT)]
            wv = [xph.tile([128, G * DH], BF16, tag=f"wv{k}", name=f"wv{k}")
                  for k in range(KT)]
            cos2 = xph.tile([128, N], BF16, tag="cos2")
            sin2 = xph.tile([128, N], BF16, tag="sin2")
            p2t = xph.tile([128, 128], BF16, tag="p2t")
            ones_col = xph.tile([128, NT, G, 1], F32, tag="ones")

            # qk weights first, then xT stripe-major; tables and the
            # projection weights slot in behind the first stripe (they are
            # first needed at rope / projection time)
            for k in range(KT):
                nc.sync.dma_start(
                    out=wqk[k],
                    in_=wqk_d.ap().rearrange("(t p) m -> t p m", p=128)[k])
            nc.vector.memset(ones_col, 1.0)
            for c in range(4):
                csl = slice(c * 512, (c + 1) * 512)
                for k in range(KT):
                    nc.sync.dma_start(
                        out=xT[k][:, csl],
                        in_=xT_d.ap().rearrange(
                            "(t p) n -> t p n", p=128)[k][:, csl])
                if c == 0:
                    nc.sync.dma_start(out=p2t, in_=p2t_d.ap())
                    nc.sync.dma_start(out=sin2, in_=sin_d.ap())
                    nc.sync.dma_start(out=cos2, in_=cos_d.ap())
                elif c == 1:
                    for k in range(KT):
                        nc.sync.dma_start(
                            out=wv[k],
                            in_=wv_d.ap().rearrange(
                                "(t p) m -> t p m", p=128)[k])
                elif c == 2:
                    for kk in range(2):
                        nc.sync.dma_start(
                            out=wout_sb[kk],
                            in_=wout_d.ap().rearrange(
                                "(t p) m -> t p m", p=128)[kk])

            def qk_chunks(m, crange):
                # m: 0,1 = q01T,q23T; 2,3 = k01T,k23T.  crange indexes 1024-
                # wide chunks (two 512 halves per PSUM tile).  pair-0 evacs on
                # the scalar engine (idle during lead-in); pair-1 evacs on DVE
                # so the scalar engine stays pure-exp during attention.
                for c2 in crange:
                    mm = ps.tile([128, 1024], F32, tag="s", name="mm_qk")
                    for half in range(2):
                        hsl = slice(half * 512, (half + 1) * 512)
                        csl = slice(c2 * 1024 + half * 512,
                                    c2 * 1024 + (half + 1) * 512)
                        for k in range(KT):
                            nc.tensor.matmul(
                                mm[:, hsl],
                                wqk[k][:, m * 128:(m + 1) * 128],
                                xT[k][:, csl],
                                start=(k == 0), stop=(k == KT - 1))
                    osl = slice(c2 * 1024, (c2 + 1) * 1024)
                    if m in (0, 2):
                        nc.scalar.copy(qk_sb[m][:, osl], mm)
                    else:
                        nc.vector.tensor_copy(qk_sb[m][:, osl], mm)

            def rope_m(m):
                tmp = rope_w.tile([128, N], BF16, tag="ropetmp")
                for c2 in range(2):
                    rot = ps.tile([128, 1024], F32, tag="s", name="mm_rot")
                    for half in range(2):
                        csl = slice(c2 * 1024 + half * 512,
                                    c2 * 1024 + (half + 1) * 512)
                        nc.tensor.matmul(
                            rot[:, half * 512:(half + 1) * 512],
                            p2t, qk_sb[m][:, csl],
                            start=True, stop=True)
                    osl = slice(c2 * 1024, (c2 + 1) * 1024)
                    nc.vector.tensor_mul(tmp[:, osl], rot, sin2[:, osl])
                nc.vector.tensor_mul(qk_sb[m], qk_sb[m], cos2)
                nc.vector.tensor_add(qk_sb[m], qk_sb[m], tmp)

            def v_tiles(trange):
                for th in trange:           # one psum tile covers 4 n-tiles
                    mm = ps.tile([128, 1024], F32, tag="s", name="mm_v")
                    for quad in range(4):
                        tn = 4 * th + quad
                        for k in range(KT):
                            nc.tensor.matmul(
                                mm[:, quad * 256:(quad + 1) * 256],
                                xT[k][:, tn * 128:(tn + 1) * 128],
                                wv[k],
                                start=(k == 0), stop=(k == KT - 1))
                    nc.vector.tensor_copy(
                        v_aug[:, 4 * th:4 * th + 4, :, 0:DH],
                        mm.rearrange("p (t h d) -> p t h d", t=4, h=G))
                if trange and trange[-1] == NT // 4 - 1:
                    nc.gpsimd.tensor_copy(v_aug[:, :, :, DH:DH + 1], ones_col)

            def attention(p, iq, filler=None):
                """One (head-pair, i-quarter of 512) block.  exp of jj runs on
                the scalar engine while the PE does PV of jj-1.  `filler`
                emits independent PE work between the last exps and the final
                PV group, covering the PE stall while the last exp drains."""
                qT = qk_sb[p]
                kTt = qk_sb[2 + p]
                i0 = iq * 512
                isl = slice(i0, i0 + 512)
                o_ps = [pso.tile([DH + 1, 512], F32, tag="o", name=f"o{hh}")
                        for hh in range(2)]

                def emit_pv(jj, exps):
                    for hh in range(2):
                        for half in range(2):
                            j = 2 * jj + half
                            nc.tensor.matmul(
                                o_ps[hh],
                                v_aug[:, j, 2 * p + hh, :],
                                exps[hh][:, half * 512:(half + 1) * 512],
                                start=(j == 0), stop=(j == NT - 1))

                pend = None
                for jj in range(NT // 2):
                    s_ps = [ps.tile([128, 1024], F32, tag="s", name=f"s{hh}")
                            for hh in range(2)]
                    for half in range(2):
                        j = 2 * jj + half
                        jsl = slice(j * 128, (j + 1) * 128)
                        for hh in range(2):
                            hsl = slice(hh * DH, (hh + 1) * DH)
                            nc.tensor.matmul(
                                s_ps[hh][:, half * 512:(half + 1) * 512],
                                kTt[hsl, jsl], qT[hsl, isl],
                                start=True, stop=True)
                    exps = []
                    for hh in range(2):
                        expT = att.tile([128, 1024], BF16, tag="exp")
                        nc.scalar.activation(expT, s_ps[hh], EXP, scale=SCALE)
                        exps.append(expT)
                    if pend is not None:
                        emit_pv(jj - 1, pend)
                    pend = exps
                if filler is not None:
                    filler()
                emit_pv(NT // 2 - 1, pend)
                return o_ps

            def att_norm(p, iq, o_ps):
                # evacuate PV accumulators so PSUM frees fast, then normalize
                # off the critical path
                isl = slice(iq * 512, (iq + 1) * 512)
                # both PSUM evacuations first: the next block's PV WAR-waits
                # on them, and the recip DMA below stalls the DVE queue
                o_sb = []
                for hh in range(2):
                    t = norm_w.tile([DH + 1, 512], F32, tag=f"osb{hh}",
                                    name=f"osb{hh}")
                    nc.vector.tensor_copy(t, o_ps[hh])
                    o_sb.append(t)
                for hh in range(2):
                    recip0 = norm_w.tile([1, 512], F32, tag=f"r0{hh}",
                                         name=f"r0{hh}")
                    nc.sync.dma_start(out=recip0, in_=o_sb[hh][DH:DH + 1, :])
                    nc.vector.reciprocal_approx_fast(recip0, recip0)
                    bc = norm_w.tile([DH, 512], F32, tag=f"bc{hh}",
                                     name=f"bc{hh}")
                    nc.gpsimd.partition_broadcast(bc, recip0)
                    if hh == 0:
                        nc.vector.tensor_mul(outT[p][0:DH, isl],
                                             o_sb[hh][0:DH, :], bc)
                    else:
                        tmpb = norm_w.tile([DH, 512], BF16, tag="tmpb")
                        nc.vector.tensor_mul(tmpb, o_sb[hh][0:DH, :], bc)
                        nc.sync.dma_start(out=outT[p][DH:2 * DH, isl],
                                          in_=tmpb)

            def proj_tile(tn):
                nsl = slice(tn * 128, (tn + 1) * 128)
                f_ps = ps.tile([128, 1024], F32, tag="s", name="f_ps")
                for c2 in range(2):
                    c2sl = slice(c2 * 512, (c2 + 1) * 512)
                    for kk in range(2):
                        nc.tensor.matmul(
                            f_ps[:, c2sl],
                            outT[kk][:, nsl], wout_sb[kk][:, c2sl],
                            start=(kk == 0), stop=(kk == 1))
                out_sb = outp.tile([128, DIM], F16, tag="osb")
                nc.vector.tensor_copy(out_sb, f_ps)
                nc.sync.dma_start(
                    out=part_d.ap().rearrange("(t p) m -> t p m", p=128)[tn],
                    in_=out_sb)

            # ---- emission order ----
            # lead-in: k01 then q01 (pair 0) + v; pair 1 QKV and the output
            # projection interleave into the attention phase.
            def att_block(p, iq, filler=None):
                att_norm(p, iq, attention(p, iq, filler))

            def projs(*tns):
                def f():
                    for tn in tns:
                        proj_tile(tn)
                return f

            # lead-in consumes xT stripe-pairs in arrival order: all work on
            # stripes (2c2, 2c2+1) is emitted before work needing the next
            # pair, so the PE never outruns the input DMA stream
            for c2 in range(2):
                qk_chunks(2, [c2])
                qk_chunks(0, [c2])
                v_tiles([2 * c2, 2 * c2 + 1])
            rope_m(2)
            rope_m(0)
            att_block(0, 0, lambda: qk_chunks(3, range(1)))
            att_block(0, 1, lambda: qk_chunks(3, range(1, 2)))
            rope_m(3)
            att_block(0, 2, lambda: qk_chunks(1, range(2)))
            rope_m(1)
            att_block(0, 3)
            # proj for i-block iq trails by one attention block so its PE
            # matmuls never wait on the just-emitted norm chain
            att_block(1, 0)
            att_block(1, 1, projs(0, 1))
            projs(2, 3)()
            att_block(1, 2, projs(4, 5))
            projs(6, 7)()
            o_last = attention(1, 3, projs(8, 9))
            att_norm(1, 3, o_last)
            projs(10, 11)()
            for tn in range(12, 16):
                proj_tile(tn)
    nc.compile()
    _cache["nc"] = nc
    return nc


def kernel(x, w_qkv, w_out, b_out, _trace=False):
    import ml_dtypes
    from concourse.bass_utils import run_bass_kernel_spmd

    x = np.asarray(x, dtype=np.float32)
    w_qkv = np.asarray(w_qkv, dtype=np.float32)
    w_out = np.asarray(w_out, dtype=np.float32)
    b_out = np.asarray(b_out, dtype=np.float32)

    cos2, sin2 = _rope_tables()
    p2t = _p2t()

    in_maps = []
    for c in range(N_CORES):
        b, g = divmod(c, G)
        cols = []
        for blk in range(2):                      # q block, k block
            base = blk * H * DH + g * G * DH
            cols.append(w_qkv[:, base:base + G * DH])
        wqk_c = np.ascontiguousarray(np.concatenate(cols, axis=1))  # [DIM, 512]
        wv_c = np.ascontiguousarray(
            w_qkv[:, 2 * H * DH + g * G * DH: 2 * H * DH + (g + 1) * G * DH])
        wout_c = np.ascontiguousarray(
            w_out[g * G * DH:(g + 1) * G * DH, :]).astype(ml_dtypes.bfloat16)
        in_maps.append({
            "xT": np.ascontiguousarray(x[b].T).astype(ml_dtypes.bfloat16),
            "wqk": wqk_c.astype(ml_dtypes.bfloat16),
            "wv": wv_c.astype(ml_dtypes.bfloat16),
            "wout": wout_c,
            "cos2": cos2.astype(ml_dtypes.bfloat16),
            "sin2": sin2.astype(ml_dtypes.bfloat16),
            "p2t": p2t.astype(ml_dtypes.bfloat16),
        })

    nc = _build()
    res = run_bass_kernel_spmd(nc, in_maps, core_ids=list(range(N_CORES)),
                               trace=_trace)
    out = np.empty((B, N, DIM), dtype=np.float32)
    for b in range(B):
        acc = res.results[G * b]["part"].astype(np.float32)
        for g in range(1, G):
            acc += res.results[G * b + g]["part"].astype(np.float32)
        out[b] = acc + b_out
    if _trace:
        kernel.last_results = res
    return out


# revision 25
# speedup vs baseline: 1.1671x; 1.0104x over previous
"""Trainium2 Bass kernel for nn_Attention_35021163332119.

Full multi-head attention: qkv = x @ w_qkv; RoPE(q, k); softmax(q k^T / sqrt(dh)) v;
out = heads @ w_out + b_out.  B=2, N=2048, DIM=1024, H=16, DH=64.

Sharding: 8 cores = (batch b in {0,1}) x (head-group g in {0..3} of 4 heads).
Each core computes its 4 heads end-to-end plus the partial output projection
for its head-group's rows of w_out; the host sums the 4 partials per batch
(in fp32, from fp16 device partials) and adds b_out.

On-core layout: x is host-transposed to xT [DIM, N] so the contraction dim
sits on SBUF partitions.  q,k are produced transposed ([dh, n], head pairs
stacked on 128 partitions) straight out of the QKV matmul; v is produced in
natural [n, dh] layout with an extra ones column, so the PV matmul (M=65)
also accumulates the softmax denominator in row 64.  RoPE's interleaved
pair-rotation is a 128x128 +/-1 permutation matmul on the PE plus DVE
multiplies against cos/sin tables.

Everything on the probability/QK side runs in bf16 (validated 7e-3 rel err
vs the 2e-2 gate): bf16 weights enable fast weight load on the PE and 2x/4x
DVE modes for the RoPE elementwise work.  Input DMA is stripe-major (weights
first, then xT in 512-column stripes across all k-tiles) so the first QKV
matmul starts as soon as ~1.5 MB has landed.  The second head-pair's QKV
matmuls and the output projection are interleaved into the attention phase
to fill PE slack under the scalar-engine exp stream.  Partial outputs DMA
out as fp16.
"""

import numpy as np

B, N, DIM, H, DH = 2, 2048, 1024, 16, 64
ROPE_BASE = 10000.0
SCALE = DH ** -0.5
N_CORES = 8
G = 4                 # heads per core
KT = DIM // 128       # contraction tiles
NT = N // 128         # sequence tiles

_cache = {}


def _rope_tables():
    inv_freq = (1.0 / (ROPE_BASE ** (np.arange(0, DH, 2, dtype=np.float32) / DH)))
    t = np.arange(N, dtype=np.float32)
    freqs = t[:, None] * inv_freq[None, :]          # [N, DH/2]
    freqs = np.repeat(freqs, 2, axis=-1)            # [N, DH] interleaved
    cosT = np.cos(freqs).T.astype(np.float32)       # [DH, N]
    sinT = np.sin(freqs).T.astype(np.float32)
    cos2 = np.concatenate([cosT, cosT], axis=0)     # [128, N] two heads stacked
    sin2 = np.concatenate([sinT, sinT], axis=0)
    return np.ascontiguousarray(cos2), np.ascontiguousarray(sin2)


def _p2t():
    # rot = P2 @ qT with P2 = blockdiag(P, P), P[2t, 2t+1] = -1, P[2t+1, 2t] = 1
    # matmul computes lhsT.T @ rhs, so pass P2.T
    p = np.zeros((DH, DH), dtype=np.float32)
    for t in range(DH // 2):
        p[2 * t, 2 * t + 1] = -1.0
        p[2 * t + 1, 2 * t] = 1.0
    p2 = np.zeros((128, 128), dtype=np.float32)
    p2[:DH, :DH] = p
    p2[DH:, DH:] = p
    return np.ascontiguousarray(p2.T)


def _build():
    if "nc" in _cache:
        return _cache["nc"]

    import concourse.mybir as mybir
    import concourse.tile as tile
    from concourse import bacc

    F32 = mybir.dt.float32
    F16 = mybir.dt.float16
    BF16 = mybir.dt.bfloat16
    EXP = mybir.ActivationFunctionType.Exp

    nc = bacc.Bacc("TRN2", target_bir_lowering=False, debug=False)
    xT_d = nc.dram_tensor("xT", [DIM, N], BF16, kind="ExternalInput")
    wqk_d = nc.dram_tensor("wqk", [DIM, 4 * 128], BF16, kind="ExternalInput")
    wv_d = nc.dram_tensor("wv", [DIM, G * DH], BF16, kind="ExternalInput")
    wout_d = nc.dram_tensor("wout", [G * DH, DIM], BF16, kind="ExternalInput")
    cos_d = nc.dram_tensor("cos2", [128, N], BF16, kind="ExternalInput")
    sin_d = nc.dram_tensor("sin2", [128, N], BF16, kind="ExternalInput")
    p2t_d = nc.dram_tensor("p2t", [128, 128], BF16, kind="ExternalInput")
    part_d = nc.dram_tensor("part", [N, DIM], F16, kind="ExternalOutput")

    with tile.TileContext(nc) as tc:
        with tc.tile_pool(name="persist", bufs=1) as persist, \
             tc.tile_pool(name="att", bufs=8) as att, \
             tc.tile_pool(name="norm_w", bufs=2) as norm_w, \
             tc.tile_pool(name="outp", bufs=3) as outp, \
             tc.tile_pool(name="xph", bufs=1) as xph, \
             tc.tile_pool(name="rope_w", bufs=2) as rope_w, \
             tc.tile_pool(name="ps", bufs=3, space="PSUM") as ps, \
             tc.tile_pool(name="pso", bufs=2, space="PSUM") as pso:

            # ---- persistent tiles ----
            qk_sb = [persist.tile([128, N], BF16, tag=f"qk{m}", name=f"qk{m}")
                     for m in range(4)]          # q01T, q23T, k01T, k23T
            v_aug = persist.tile([128, NT, G, DH + 1], BF16, tag="vaug")
            wout_sb = [persist.tile([128, DIM], BF16, tag=f"wo{kk}", name=f"wo{kk}")
                       for kk in range(2)]
            outT = [persist.tile([128, N], BF16, tag=f"outT{p}", name=f"outT{p}")
                    for p in range(2)]

            # ---- phase-1 tiles ----
            xT = [xph.tile([128, N], BF16, tag=f"xT{k}", name=f"xT{k}")
                  for k in range(KT)]
            wqk = [xph.tile([128, 4 * 128], BF16, tag=f"wqk{k}", name=f"wqk{k}")
                   for k in range(KT)]
            wv = [xph.tile([128, G * DH], BF16, tag=f"wv{k}", name=f"wv{k}")
                  for k in range(KT)]
            cos2 = xph.tile([128, N], BF16, tag="cos2")
            sin2 = xph.tile([128, N], BF16, tag="sin2")
            p2t = xph.tile([128, 128], BF16, tag="p2t")
            ones_col = xph.tile([128, NT, G, 1], F32, tag="ones")

            # qk weights first, then xT stripe-major; tables and the
            # projection weights slot in behind the first stripe (they are
            # first needed at rope / projection time)
            for k in range(KT):
                nc.sync.dma_start(
                    out=wqk[k],
                    in_=wqk_d.ap().rearrange("(t p) m -> t p m", p=128)[k])
            nc.vector.memset(ones_col, 1.0)
            for c in range(4):
                csl = slice(c * 512, (c + 1) * 512)
                for k in range(KT):
                    nc.sync.dma_start(
                        out=xT[k][:, csl],
                        in_=xT_d.ap().rearrange(
                            "(t p) n -> t p n", p=128)[k][:, csl])
                if c == 0:
                    nc.sync.dma_start(out=p2t, in_=p2t_d.ap())
                    for k in range(KT):
                        nc.sync.dma_start(
                            out=wv[k],
                            in_=wv_d.ap().rearrange(
                                "(t p) m -> t p m", p=128)[k])
                    nc.sync.dma_start(out=sin2, in_=sin_d.ap())
                elif c == 1:
                    nc.sync.dma_start(out=cos2, in_=cos_d.ap())
                elif c == 2:
                    for kk in range(2):
                        nc.sync.dma_start(
                            out=wout_sb[kk],
                            in_=wout_d.ap().rearrange(
                                "(t p) m -> t p m", p=128)[kk])

            def qk_chunks(m, crange):
                # m: 0,1 = q01T,q23T; 2,3 = k01T,k23T.  crange indexes 1024-
                # wide chunks (two 512 halves per PSUM tile).  pair-0 evacs on
                # the scalar engine (idle during lead-in); pair-1 evacs on DVE
                # so the scalar engine stays pure-exp during attention.
                for c2 in crange:
                    mm = ps.tile([128, 1024], F32, tag="s", name="mm_qk")
                    for half in range(2):
                        hsl = slice(half * 512, (half + 1) * 512)
                        csl = slice(c2 * 1024 + half * 512,
                                    c2 * 1024 + (half + 1) * 512)
                        for k in range(KT):
                            nc.tensor.matmul(
                                mm[:, hsl],
                                wqk[k][:, m * 128:(m + 1) * 128],
                                xT[k][:, csl],
                                start=(k == 0), stop=(k == KT - 1))
                    osl = slice(c2 * 1024, (c2 + 1) * 1024)
                    if m in (0, 2):
                        nc.scalar.copy(qk_sb[m][:, osl], mm)
                    else:
                        nc.vector.tensor_copy(qk_sb[m][:, osl], mm)

            rope_tmp = {}

            def rope_rot(m, c2):
                if m not in rope_tmp:
                    rope_tmp[m] = rope_w.tile([128, N], BF16,
                                              tag=f"ropetmp{m % 2}",
                                              name=f"ropetmp{m}")
                rot = ps.tile([128, 1024], F32, tag="s", name="mm_rot")
                for half in range(2):
                    csl = slice(c2 * 1024 + half * 512,
                                c2 * 1024 + (half + 1) * 512)
                    nc.tensor.matmul(
                        rot[:, half * 512:(half + 1) * 512],
                        p2t, qk_sb[m][:, csl],
                        start=True, stop=True)
                osl = slice(c2 * 1024, (c2 + 1) * 1024)
                nc.vector.tensor_mul(rope_tmp[m][:, osl], rot, sin2[:, osl])

            def rope_fin(m):
                nc.vector.tensor_mul(qk_sb[m], qk_sb[m], cos2)
                nc.vector.tensor_add(qk_sb[m], qk_sb[m], rope_tmp[m])
                del rope_tmp[m]

            def rope_m(m):
                for c2 in range(2):
                    rope_rot(m, c2)
                rope_fin(m)

            def v_tiles(trange):
                for th in trange:           # one psum tile covers 4 n-tiles
                    mm = ps.tile([128, 1024], F32, tag="s", name="mm_v")
                    for quad in range(4):
                        tn = 4 * th + quad
                        for k in range(KT):
                            nc.tensor.matmul(
                                mm[:, quad * 256:(quad + 1) * 256],
                                xT[k][:, tn * 128:(tn + 1) * 128],
                                wv[k],
                                start=(k == 0), stop=(k == KT - 1))
                    nc.vector.tensor_copy(
                        v_aug[:, 4 * th:4 * th + 4, :, 0:DH],
                        mm.rearrange("p (t h d) -> p t h d", t=4, h=G))
                if trange and trange[-1] == NT // 4 - 1:
                    nc.gpsimd.tensor_copy(v_aug[:, :, :, DH:DH + 1], ones_col)

            def attention(p, iq, filler=None):
                """One (head-pair, i-quarter of 512) block.  exp of jj runs on
                the scalar engine while the PE does PV of jj-1.  `filler`
                emits independent PE work between the last exps and the final
                PV group, covering the PE stall while the last exp drains."""
                qT = qk_sb[p]
                kTt = qk_sb[2 + p]
                i0 = iq * 512
                isl = slice(i0, i0 + 512)
                o_ps = [pso.tile([DH + 1, 512], F32, tag="o", name=f"o{hh}")
                        for hh in range(2)]

                def emit_pv(jj, exps):
                    for hh in range(2):
                        for half in range(2):
                            j = 2 * jj + half
                            nc.tensor.matmul(
                                o_ps[hh],
                                v_aug[:, j, 2 * p + hh, :],
                                exps[hh][:, half * 512:(half + 1) * 512],
                                start=(j == 0), stop=(j == NT - 1))

                pend = None
                for jj in range(NT // 2):
                    s_ps = [ps.tile([128, 1024], F32, tag="s", name=f"s{hh}")
                            for hh in range(2)]
                    for half in range(2):
                        j = 2 * jj + half
                        jsl = slice(j * 128, (j + 1) * 128)
                        for hh in range(2):
                            hsl = slice(hh * DH, (hh + 1) * DH)
                            nc.tensor.matmul(
                                s_ps[hh][:, half * 512:(half + 1) * 512],
                                kTt[hsl, jsl], qT[hsl, isl],
                                start=True, stop=True)
                    exps = []
                    for hh in range(2):
                        expT = att.tile([128, 1024], BF16, tag="exp")
                        nc.scalar.activation(expT, s_ps[hh], EXP, scale=SCALE)
                        exps.append(expT)
                    if pend is not None:
                        emit_pv(jj - 1, pend)
                    pend = exps
                if filler is not None:
                    filler()
                emit_pv(NT // 2 - 1, pend)
                return o_ps

            def att_norm(p, iq, o_ps):
                # evacuate PV accumulators so PSUM frees fast, then normalize
                # off the critical path
                isl = slice(iq * 512, (iq + 1) * 512)
                # both PSUM evacuations first: the next block's PV WAR-waits
                # on them, and the recip DMA below stalls the DVE queue
                o_sb = []
                for hh in range(2):
                    t = norm_w.tile([DH + 1, 512], F32, tag=f"osb{hh}",
                                    name=f"osb{hh}")
                    nc.vector.tensor_copy(t, o_ps[hh])
                    o_sb.append(t)
                for hh in range(2):
                    recip0 = norm_w.tile([1, 512], F32, tag=f"r0{hh}",
                                         name=f"r0{hh}")
                    nc.sync.dma_start(out=recip0, in_=o_sb[hh][DH:DH + 1, :])
                    nc.vector.reciprocal_approx_fast(recip0, recip0)
                    bc = norm_w.tile([DH, 512], F32, tag=f"bc{hh}",
                                     name=f"bc{hh}")
                    nc.gpsimd.partition_broadcast(bc, recip0)
                    if hh == 0:
                        nc.vector.tensor_mul(outT[p][0:DH, isl],
                                             o_sb[hh][0:DH, :], bc)
                    else:
                        tmpb = norm_w.tile([DH, 512], BF16, tag="tmpb")
                        nc.vector.tensor_mul(tmpb, o_sb[hh][0:DH, :], bc)
                        nc.sync.dma_start(out=outT[p][DH:2 * DH, isl],
                                          in_=tmpb)

            def proj_tile(tn):
                nsl = slice(tn * 128, (tn + 1) * 128)
                f_ps = ps.tile([128, 1024], F32, tag="s", name="f_ps")
                for c2 in range(2):
                    c2sl = slice(c2 * 512, (c2 + 1) * 512)
                    for kk in range(2):
                        nc.tensor.matmul(
                            f_ps[:, c2sl],
                            outT[kk][:, nsl], wout_sb[kk][:, c2sl],
                            start=(kk == 0), stop=(kk == 1))
                out_sb = outp.tile([128, DIM], F16, tag="osb")
                nc.vector.tensor_copy(out_sb, f_ps)
                nc.sync.dma_start(
                    out=part_d.ap().rearrange("(t p) m -> t p m", p=128)[tn],
                    in_=out_sb)

            # ---- emission order ----
            # lead-in: k01 then q01 (pair 0) + v; pair 1 QKV and the output
            # projection interleave into the attention phase.
            def att_block(p, iq, filler=None):
                att_norm(p, iq, attention(p, iq, filler))

            def projs(*tns):
                def f():
                    for tn in tns:
                        proj_tile(tn)
                return f

            # lead-in consumes xT stripe-pairs in arrival order: all work on
            # stripes (2c2, 2c2+1) is emitted before work needing the next
            # pair, so the PE never outruns the input DMA stream; the rot
            # matmuls slot in per chunk so the DVE RoPE chain overlaps the PE
            # instead of serializing at the end
            for c2 in range(2):
                qk_chunks(2, [c2])
                qk_chunks(0, [c2])
                rope_rot(2, c2)
                v_tiles([2 * c2])
                rope_rot(0, c2)
                v_tiles([2 * c2 + 1])
            rope_fin(2)
            rope_fin(0)
            att_block(0, 0, lambda: qk_chunks(3, [0]))
            att_block(0, 1, lambda: (qk_chunks(3, [1]), rope_rot(3, 0)))
            att_block(0, 2, lambda: (qk_chunks(1, [0]), qk_chunks(1, [1]),
                                     rope_rot(3, 1), rope_fin(3)))
            att_block(0, 3, lambda: (rope_rot(1, 0), rope_rot(1, 1),
                                     rope_fin(1)))
            # proj for i-block iq trails by one attention block so its PE
            # matmuls never wait on the just-emitted norm chain
            att_block(1, 0)
            att_block(1, 1, projs(0, 1))
            projs(2, 3)()
            att_block(1, 2, projs(4, 5))
            projs(6, 7)()
            o_last = attention(1, 3, projs(8, 9))
            att_norm(1, 3, o_last)
            projs(10, 11)()
            for tn in range(12, 16):
                proj_tile(tn)
    nc.compile()
    _cache["nc"] = nc
    return nc


def kernel(x, w_qkv, w_out, b_out, _trace=False):
    import ml_dtypes
    from concourse.bass_utils import run_bass_kernel_spmd

    x = np.asarray(x, dtype=np.float32)
    w_qkv = np.asarray(w_qkv, dtype=np.float32)
    w_out = np.asarray(w_out, dtype=np.float32)
    b_out = np.asarray(b_out, dtype=np.float32)

    cos2, sin2 = _rope_tables()
    p2t = _p2t()

    in_maps = []
    for c in range(N_CORES):
        b, g = divmod(c, G)
        cols = []
        for blk in range(2):                      # q block, k block
            base = blk * H * DH + g * G * DH
            cols.append(w_qkv[:, base:base + G * DH])
        wqk_c = np.ascontiguousarray(np.concatenate(cols, axis=1))  # [DIM, 512]
        wv_c = np.ascontiguousarray(
            w_qkv[:, 2 * H * DH + g * G * DH: 2 * H * DH + (g + 1) * G * DH])
        wout_c = np.ascontiguousarray(
            w_out[g * G * DH:(g + 1) * G * DH, :]).astype(ml_dtypes.bfloat16)
        in_maps.append({
            "xT": np.ascontiguousarray(x[b].T).astype(ml_dtypes.bfloat16),
            "wqk": wqk_c.astype(ml_dtypes.bfloat16),
            "wv": wv_c.astype(ml_dtypes.bfloat16),
            "wout": wout_c,
            "cos2": cos2.astype(ml_dtypes.bfloat16),
            "sin2": sin2.astype(ml_dtypes.bfloat16),
            "p2t": p2t.astype(ml_dtypes.bfloat16),
        })

    nc = _build()
    res = run_bass_kernel_spmd(nc, in_maps, core_ids=list(range(N_CORES)),
                               trace=_trace)
    out = np.empty((B, N, DIM), dtype=np.float32)
    for b in range(B):
        acc = res.results[G * b]["part"].astype(np.float32)
        for g in range(1, G):
            acc += res.results[G * b + g]["part"].astype(np.float32)
        out[b] = acc + b_out
    if _trace:
        kernel.last_results = res
    return out


# revision 26
# speedup vs baseline: 1.1876x; 1.0175x over previous
"""Trainium2 Bass kernel for nn_Attention_35021163332119.

Full multi-head attention: qkv = x @ w_qkv; RoPE(q, k); softmax(q k^T / sqrt(dh)) v;
out = heads @ w_out + b_out.  B=2, N=2048, DIM=1024, H=16, DH=64.

Sharding: 8 cores = (batch b in {0,1}) x (head-group g in {0..3} of 4 heads).
Each core computes its 4 heads end-to-end plus the partial output projection
for its head-group's rows of w_out; the host sums the 4 partials per batch
(in fp32, from fp16 device partials) and adds b_out.

On-core layout: x is host-transposed to xT [DIM, N] so the contraction dim
sits on SBUF partitions.  q,k are produced transposed ([dh, n], head pairs
stacked on 128 partitions) straight out of the QKV matmul; v is produced in
natural [n, dh] layout with an extra ones column, so the PV matmul (M=65)
also accumulates the softmax denominator in row 64.  RoPE's interleaved
pair-rotation is a 128x128 +/-1 permutation matmul on the PE plus DVE
multiplies against cos/sin tables.

Everything on the probability/QK side runs in bf16 (validated 7e-3 rel err
vs the 2e-2 gate): bf16 weights enable fast weight load on the PE and 2x/4x
DVE modes for the RoPE elementwise work.  Input DMA is stripe-major (weights
first, then xT in 512-column stripes across all k-tiles) so the first QKV
matmul starts as soon as ~1.5 MB has landed.  The second head-pair's QKV
matmuls and the output projection are interleaved into the attention phase
to fill PE slack under the scalar-engine exp stream.  Partial outputs DMA
out as fp16.
"""

import numpy as np

B, N, DIM, H, DH = 2, 2048, 1024, 16, 64
ROPE_BASE = 10000.0
SCALE = DH ** -0.5
N_CORES = 8
G = 4                 # heads per core
KT = DIM // 128       # contraction tiles
NT = N // 128         # sequence tiles

_cache = {}


def _rope_tables():
    inv_freq = (1.0 / (ROPE_BASE ** (np.arange(0, DH, 2, dtype=np.float32) / DH)))
    t = np.arange(N, dtype=np.float32)
    freqs = t[:, None] * inv_freq[None, :]          # [N, DH/2]
    freqs = np.repeat(freqs, 2, axis=-1)            # [N, DH] interleaved
    cosT = np.cos(freqs).T.astype(np.float32)       # [DH, N]
    sinT = np.sin(freqs).T.astype(np.float32)
    cos2 = np.concatenate([cosT, cosT], axis=0)     # [128, N] two heads stacked
    sin2 = np.concatenate([sinT, sinT], axis=0)
    return np.ascontiguousarray(cos2), np.ascontiguousarray(sin2)


def _p2t():
    # rot = P2 @ qT with P2 = blockdiag(P, P), P[2t, 2t+1] = -1, P[2t+1, 2t] = 1
    # matmul computes lhsT.T @ rhs, so pass P2.T
    p = np.zeros((DH, DH), dtype=np.float32)
    for t in range(DH // 2):
        p[2 * t, 2 * t + 1] = -1.0
        p[2 * t + 1, 2 * t] = 1.0
    p2 = np.zeros((128, 128), dtype=np.float32)
    p2[:DH, :DH] = p
    p2[DH:, DH:] = p
    return np.ascontiguousarray(p2.T)


def _build():
    if "nc" in _cache:
        return _cache["nc"]

    import concourse.mybir as mybir
    import concourse.tile as tile
    from concourse import bacc

    F32 = mybir.dt.float32
    F16 = mybir.dt.float16
    BF16 = mybir.dt.bfloat16
    EXP = mybir.ActivationFunctionType.Exp

    nc = bacc.Bacc("TRN2", target_bir_lowering=False, debug=False)
    xT_d = nc.dram_tensor("xT", [DIM, N], BF16, kind="ExternalInput")
    wqk_d = nc.dram_tensor("wqk", [DIM, 4 * 128], BF16, kind="ExternalInput")
    wv_d = nc.dram_tensor("wv", [DIM, G * DH], BF16, kind="ExternalInput")
    wout_d = nc.dram_tensor("wout", [G * DH, DIM], BF16, kind="ExternalInput")
    cos_d = nc.dram_tensor("cos2", [128, N], BF16, kind="ExternalInput")
    sin_d = nc.dram_tensor("sin2", [128, N], BF16, kind="ExternalInput")
    p2t_d = nc.dram_tensor("p2t", [128, 128], BF16, kind="ExternalInput")
    part_d = nc.dram_tensor("part", [N, DIM], F16, kind="ExternalOutput")

    with tile.TileContext(nc) as tc:
        with tc.tile_pool(name="persist", bufs=1) as persist, \
             tc.tile_pool(name="att", bufs=8) as att, \
             tc.tile_pool(name="norm_w", bufs=2) as norm_w, \
             tc.tile_pool(name="outp", bufs=3) as outp, \
             tc.tile_pool(name="xph", bufs=1) as xph, \
             tc.tile_pool(name="rope_w", bufs=2) as rope_w, \
             tc.tile_pool(name="ps", bufs=3, space="PSUM") as ps, \
             tc.tile_pool(name="pso", bufs=2, space="PSUM") as pso:

            # ---- persistent tiles ----
            qk_sb = [persist.tile([128, N], BF16, tag=f"qk{m}", name=f"qk{m}")
                     for m in range(4)]          # q01T, q23T, k01T, k23T
            v_aug = persist.tile([128, NT, G, DH + 1], BF16, tag="vaug")
            wout_sb = [persist.tile([128, DIM], BF16, tag=f"wo{kk}", name=f"wo{kk}")
                       for kk in range(2)]
            outT = [persist.tile([128, N], BF16, tag=f"outT{p}", name=f"outT{p}")
                    for p in range(2)]

            # ---- phase-1 tiles ----
            xT = [xph.tile([128, N], BF16, tag=f"xT{k}", name=f"xT{k}")
                  for k in range(KT)]
            wqk = [xph.tile([128, 4 * 128], BF16, tag=f"wqk{k}", name=f"wqk{k}")
                   for k in range(KT)]
            wv = [xph.tile([128, G * DH], BF16, tag=f"wv{k}", name=f"wv{k}")
                  for k in range(KT)]
            cos2 = xph.tile([128, N], BF16, tag="cos2")
            sin2 = xph.tile([128, N], BF16, tag="sin2")
            p2t = xph.tile([128, 128], BF16, tag="p2t")
            ones_col = xph.tile([128, NT, G, 1], F32, tag="ones")

            # qk weights first, then xT stripe-major; tables and the
            # projection weights slot in behind the first stripe (they are
            # first needed at rope / projection time)
            for k in range(KT):
                nc.sync.dma_start(
                    out=wqk[k],
                    in_=wqk_d.ap().rearrange("(t p) m -> t p m", p=128)[k])
            nc.vector.memset(ones_col, 1.0)
            for c in range(4):
                csl = slice(c * 512, (c + 1) * 512)
                for k in range(KT):
                    nc.sync.dma_start(
                        out=xT[k][:, csl],
                        in_=xT_d.ap().rearrange(
                            "(t p) n -> t p n", p=128)[k][:, csl])
                if c == 0:
                    nc.sync.dma_start(out=p2t, in_=p2t_d.ap())
                    for k in range(KT):
                        nc.sync.dma_start(
                            out=wv[k],
                            in_=wv_d.ap().rearrange(
                                "(t p) m -> t p m", p=128)[k])
                    nc.sync.dma_start(out=sin2, in_=sin_d.ap())
                elif c == 1:
                    nc.sync.dma_start(out=cos2, in_=cos_d.ap())
                elif c == 2:
                    for kk in range(2):
                        nc.sync.dma_start(
                            out=wout_sb[kk],
                            in_=wout_d.ap().rearrange(
                                "(t p) m -> t p m", p=128)[kk])

            def qk_chunks(m, crange):
                # m: 0,1 = q01T,q23T; 2,3 = k01T,k23T.  crange indexes 1024-
                # wide chunks (two 512 halves per PSUM tile).  pair-0 evacs on
                # the scalar engine (idle during lead-in); pair-1 evacs on DVE
                # so the scalar engine stays pure-exp during attention.
                for c2 in crange:
                    mm = ps.tile([128, 1024], F32, tag="s", name="mm_qk")
                    for half in range(2):
                        hsl = slice(half * 512, (half + 1) * 512)
                        csl = slice(c2 * 1024 + half * 512,
                                    c2 * 1024 + (half + 1) * 512)
                        for k in range(KT):
                            nc.tensor.matmul(
                                mm[:, hsl],
                                wqk[k][:, m * 128:(m + 1) * 128],
                                xT[k][:, csl],
                                start=(k == 0), stop=(k == KT - 1))
                    osl = slice(c2 * 1024, (c2 + 1) * 1024)
                    if m in (0, 2):
                        nc.scalar.copy(qk_sb[m][:, osl], mm)
                    else:
                        nc.vector.tensor_copy(qk_sb[m][:, osl], mm)

            rope_tmp = {}

            def rope_rot(m, c2):
                if m not in rope_tmp:
                    rope_tmp[m] = rope_w.tile([128, N], BF16,
                                              tag=f"ropetmp{m % 2}",
                                              name=f"ropetmp{m}")
                rot = ps.tile([128, 1024], F32, tag="s", name="mm_rot")
                for half in range(2):
                    csl = slice(c2 * 1024 + half * 512,
                                c2 * 1024 + (half + 1) * 512)
                    nc.tensor.matmul(
                        rot[:, half * 512:(half + 1) * 512],
                        p2t, qk_sb[m][:, csl],
                        start=True, stop=True)
                osl = slice(c2 * 1024, (c2 + 1) * 1024)
                nc.vector.tensor_mul(rope_tmp[m][:, osl], rot, sin2[:, osl])

            def rope_fin(m):
                nc.vector.tensor_mul(qk_sb[m], qk_sb[m], cos2)
                nc.vector.tensor_add(qk_sb[m], qk_sb[m], rope_tmp[m])
                del rope_tmp[m]

            def rope_m(m):
                for c2 in range(2):
                    rope_rot(m, c2)
                rope_fin(m)

            def v_tiles(trange):
                for th in trange:           # one psum tile covers 4 n-tiles
                    mm = ps.tile([128, 1024], F32, tag="s", name="mm_v")
                    for quad in range(4):
                        tn = 4 * th + quad
                        for k in range(KT):
                            nc.tensor.matmul(
                                mm[:, quad * 256:(quad + 1) * 256],
                                xT[k][:, tn * 128:(tn + 1) * 128],
                                wv[k],
                                start=(k == 0), stop=(k == KT - 1))
                    nc.vector.tensor_copy(
                        v_aug[:, 4 * th:4 * th + 4, :, 0:DH],
                        mm.rearrange("p (t h d) -> p t h d", t=4, h=G))
                if trange and trange[-1] == NT // 4 - 1:
                    nc.gpsimd.tensor_copy(v_aug[:, :, :, DH:DH + 1], ones_col)

            def attention(p, iq, filler=None):
                """One (head-pair, i-quarter of 512) block.  exp of jj runs on
                the scalar engine while the PE does PV of jj-1.  `filler`
                emits independent PE work between the last exps and the final
                PV group, covering the PE stall while the last exp drains."""
                qT = qk_sb[p]
                kTt = qk_sb[2 + p]
                i0 = iq * 512
                isl = slice(i0, i0 + 512)
                o_ps = [pso.tile([DH + 1, 512], F32, tag="o", name=f"o{hh}")
                        for hh in range(2)]

                def emit_pv(jj, exps):
                    for hh in range(2):
                        for half in range(2):
                            j = 2 * jj + half
                            nc.tensor.matmul(
                                o_ps[hh],
                                v_aug[:, j, 2 * p + hh, :],
                                exps[hh][:, half * 512:(half + 1) * 512],
                                start=(j == 0), stop=(j == NT - 1))

                pend = None
                for jj in range(NT // 2):
                    s_ps = [ps.tile([128, 1024], F32, tag="s", name=f"s{hh}")
                            for hh in range(2)]
                    for half in range(2):
                        j = 2 * jj + half
                        jsl = slice(j * 128, (j + 1) * 128)
                        for hh in range(2):
                            hsl = slice(hh * DH, (hh + 1) * DH)
                            nc.tensor.matmul(
                                s_ps[hh][:, half * 512:(half + 1) * 512],
                                kTt[hsl, jsl], qT[hsl, isl],
                                start=True, stop=True)
                    exps = []
                    for hh in range(2):
                        expT = att.tile([128, 1024], BF16, tag="exp")
                        nc.scalar.activation(expT, s_ps[hh], EXP, scale=SCALE)
                        exps.append(expT)
                    if pend is not None:
                        emit_pv(jj - 1, pend)
                    pend = exps
                if filler is not None:
                    filler()
                emit_pv(NT // 2 - 1, pend)
                return o_ps

            def att_norm(p, iq, o_ps):
                # evacuate PV accumulators so PSUM frees fast, then normalize
                # off the critical path
                isl = slice(iq * 512, (iq + 1) * 512)
                # both PSUM evacuations first: the next block's PV WAR-waits
                # on them, and the recip DMA below stalls the DVE queue
                o_sb = []
                for hh in range(2):
                    t = norm_w.tile([DH + 1, 512], F32, tag=f"osb{hh}",
                                    name=f"osb{hh}")
                    nc.vector.tensor_copy(t, o_ps[hh])
                    o_sb.append(t)
                for hh in range(2):
                    recip0 = norm_w.tile([1, 512], F32, tag=f"r0{hh}",
                                         name=f"r0{hh}")
                    nc.sync.dma_start(out=recip0, in_=o_sb[hh][DH:DH + 1, :])
                    nc.vector.reciprocal_approx_fast(recip0, recip0)
                    bc = norm_w.tile([DH, 512], F32, tag=f"bc{hh}",
                                     name=f"bc{hh}")
                    nc.gpsimd.partition_broadcast(bc, recip0)
                    if hh == 0:
                        nc.vector.tensor_mul(outT[p][0:DH, isl],
                                             o_sb[hh][0:DH, :], bc)
                    else:
                        tmpb = norm_w.tile([DH, 512], BF16, tag="tmpb")
                        nc.vector.tensor_mul(tmpb, o_sb[hh][0:DH, :], bc)
                        nc.sync.dma_start(out=outT[p][DH:2 * DH, isl],
                                          in_=tmpb)

            def proj_tile(tn):
                nsl = slice(tn * 128, (tn + 1) * 128)
                f_ps = ps.tile([128, 1024], F32, tag="s", name="f_ps")
                for c2 in range(2):
                    c2sl = slice(c2 * 512, (c2 + 1) * 512)
                    for kk in range(2):
                        nc.tensor.matmul(
                            f_ps[:, c2sl],
                            outT[kk][:, nsl], wout_sb[kk][:, c2sl],
                            start=(kk == 0), stop=(kk == 1))
                out_sb = outp.tile([128, DIM], F16, tag="osb")
                nc.vector.tensor_copy(out_sb, f_ps)
                nc.sync.dma_start(
                    out=part_d.ap().rearrange("(t p) m -> t p m", p=128)[tn],
                    in_=out_sb)

            # ---- emission order ----
            # lead-in: k01 then q01 (pair 0) + v; pair 1 QKV and the output
            # projection interleave into the attention phase.
            def att_block(p, iq, filler=None):
                att_norm(p, iq, attention(p, iq, filler))

            def projs(*tns):
                def f():
                    for tn in tns:
                        proj_tile(tn)
                return f

            # lead-in consumes xT stripe-pairs in arrival order: all work on
            # stripes (2c2, 2c2+1) is emitted before work needing the next
            # pair, so the PE never outruns the input DMA stream; the rot
            # matmuls slot in per chunk so the DVE RoPE chain overlaps the PE
            # instead of serializing at the end
            for c2 in range(2):
                qk_chunks(2, [c2])
                qk_chunks(0, [c2])
                rope_rot(2, c2)
                v_tiles([2 * c2])
                rope_rot(0, c2)
                v_tiles([2 * c2 + 1])
            rope_fin(2)
            rope_fin(0)
            att_block(0, 0, lambda: qk_chunks(3, [0]))
            att_block(0, 1, lambda: (qk_chunks(3, [1]), rope_rot(3, 0),
                                     qk_chunks(1, [0])))
            att_block(0, 2, lambda: (qk_chunks(1, [1]), rope_rot(3, 1),
                                     rope_fin(3), rope_rot(1, 0)))
            rope_rot(1, 1)
            rope_fin(1)
            att_block(0, 3)
            # proj for i-block iq trails by one attention block so its PE
            # matmuls never wait on the just-emitted norm chain
            att_block(1, 0)
            att_block(1, 1, projs(0, 1))
            projs(2, 3)()
            att_block(1, 2, projs(4, 5))
            projs(6, 7)()
            o_last = attention(1, 3, projs(8, 9))
            att_norm(1, 3, o_last)
            projs(10, 11)()
            for tn in range(12, 16):
                proj_tile(tn)
    nc.compile()
    _cache["nc"] = nc
    return nc


def kernel(x, w_qkv, w_out, b_out, _trace=False):
    import ml_dtypes
    from concourse.bass_utils import run_bass_kernel_spmd

    x = np.asarray(x, dtype=np.float32)
    w_qkv = np.asarray(w_qkv, dtype=np.float32)
    w_out = np.asarray(w_out, dtype=np.float32)
    b_out = np.asarray(b_out, dtype=np.float32)

    cos2, sin2 = _rope_tables()
    p2t = _p2t()

    in_maps = []
    for c in range(N_CORES):
        b, g = divmod(c, G)
        cols = []
        for blk in range(2):                      # q block, k block
            base = blk * H * DH + g * G * DH
            cols.append(w_qkv[:, base:base + G * DH])
        wqk_c = np.ascontiguousarray(np.concatenate(cols, axis=1))  # [DIM, 512]
        wv_c = np.ascontiguousarray(
            w_qkv[:, 2 * H * DH + g * G * DH: 2 * H * DH + (g + 1) * G * DH])
        wout_c = np.ascontiguousarray(
            w_out[g * G * DH:(g + 1) * G * DH, :]).astype(ml_dtypes.bfloat16)
        in_maps.append({
            "xT": np.ascontiguousarray(x[b].T).astype(ml_dtypes.bfloat16),
            "wqk": wqk_c.astype(ml_dtypes.bfloat16),
            "wv": wv_c.astype(ml_dtypes.bfloat16),
            "wout": wout_c,
            "cos2": cos2.astype(ml_dtypes.bfloat16),
            "sin2": sin2.astype(ml_dtypes.bfloat16),
            "p2t": p2t.astype(ml_dtypes.bfloat16),
        })

    nc = _build()
    res = run_bass_kernel_spmd(nc, in_maps, core_ids=list(range(N_CORES)),
                               trace=_trace)
    out = np.empty((B, N, DIM), dtype=np.float32)
    for b in range(B):
        acc = res.results[G * b]["part"].astype(np.float32)
        for g in range(1, G):
            acc += res.results[G * b + g]["part"].astype(np.float32)
        out[b] = acc + b_out
    if _trace:
        kernel.last_results = res
    return out
